# revision 47
# baseline (speedup 1.0000x reference)
"""Trainium2 Bass kernel for nn_AttentionAggregator (GNN message passing).

out = norm(h0)*scale0+offset0 + norm(agg)*scale1+offset1
  h0 = relu(x@W0.T + b0); h1 = relu(x@W1.T + b1)
  a_self = lrelu(h0@att[:d]); a_neigh = lrelu(h1@att[d:])
  agg[i] = sum_{(i,j) in E} (a_self[i]+a_neigh[j]) * h1[j]

Strategy (8 cores, SPMD, no collectives):
  - nodes (rows of output) sharded across cores; edges partitioned by dest row
  - every core recomputes full h1 and writes a node-major payload
    row[j] = [h1[j] | a_neigh[j]*h1[j]] (256 bf16 = 512 B) to its HBM
  - payload build is pipelined per ascending source segment with the
    per-dest-tile dma_gather + one-hot PE segment-sum; gathers alternate
    4 SWDGE queues to overlap descriptor gen with ring drain; per-core
    exact counts via reg_load (trimmed tail idx pads MUST be -1)
  - agg partials accumulate in SBUF f32 across segments; the epilogue
    recomputes h0 per tile and fuses both norms
"""

import hashlib
from contextlib import ExitStack

import numpy as np
import ml_dtypes

import concourse.bass as bass
import concourse.bacc as bacc
import concourse.tile as tile
import concourse.mybir as mybir
from concourse import bass_utils
from concourse import library_config

BF16 = mybir.dt.bfloat16
F32 = mybir.dt.float32
I16 = mybir.dt.int16
I32 = mybir.dt.int32

D = 128  # feature dim (both in and out)
P = 128  # partitions


class Cfg:
    def __init__(self, n_nodes, n_cores):
        assert n_nodes % n_cores == 0
        self.N = n_nodes
        self.M = n_cores
        self.NC = n_nodes // n_cores          # dest rows per core
        self.T = (self.NC + P - 1) // P       # dest tiles per core
        self.NC_PAD = self.T * P
        self.SB = 512                         # phase-1 node superblock
        self.NB = (n_nodes + self.SB - 1) // self.SB
        self.NPAD = self.NB * self.SB
        # ascending source segments (rows); each <= 32768 (int16 idx range),
        # each a multiple of SB. A tiny first segment lets the gathers start
        # almost immediately while later payload builds hide under them.
        sizes = []
        rem = self.NPAD
        for want in (2048, 8192):
            if rem > want * 2:
                sizes.append(want)
                rem -= want
        # near-equal tail segments (512-mult), no runt remainder
        ntail = (rem + 12288 - 1) // 12288
        base = (rem // ntail) // self.SB * self.SB
        extra = (rem - ntail * base) // self.SB
        for i in range(ntail):
            sizes.append(base + self.SB * (1 if i < extra else 0))
        assert sum(sizes) == self.NPAD
        self.SEG_SIZES = sizes
        self.SEG_STARTS = np.concatenate(([0], np.cumsum(sizes))).astype(np.int64)
        self.NSEG = len(sizes)
        self.SBSEG = [sz // self.SB for sz in sizes]  # superblocks per segment
        # filled by prep():
        self.B = None        # [NSEG][T] bucket layout sizes (16-mult, shared)
        self.CHUNKS = None   # total chunks
        self.IDXC = None     # total idx columns (sum B / 16)
        self.IDXC_SEG = None # [NSEG] idx columns per segment
        self.G = None        # NSEG*T gather count


def _prep_edges(cfg: Cfg, row, col):
    """Sort edges per core into (segment, tile) buckets; build tables."""
    M, NC, T, NSEG = cfg.M, cfg.NC, cfg.T, cfg.NSEG
    row = np.asarray(row).astype(np.int64)
    col = np.asarray(col).astype(np.int64)
    seg_starts = cfg.SEG_STARTS

    per_core = []
    counts = np.zeros((M, NSEG, T), dtype=np.int64)
    for m in range(M):
        mask = (row >= m * NC) & (row < (m + 1) * NC)
        r = row[mask] - m * NC
        c = col[mask]
        t = r >> 7
        s = np.searchsorted(seg_starts, c, side="right") - 1
        order = np.lexsort((c, t, s))
        r, c, t, s = r[order], c[order], t[order], s[order]
        np.add.at(counts[m], (s, t), 1)
        per_core.append((r, c, t, s))

    cnt16 = ((counts + 15) // 16) * 16            # per-core 16-mult counts
    B = cnt16.max(axis=0)                          # [NSEG, T] shared layout
    CH = (B + P - 1) // P                          # chunks per bucket
    CHUNKS = int(CH.sum())
    TOT = int(B.sum())
    IDXC = TOT // 16
    IDXC_SEG = (B.sum(axis=1) // 16).astype(np.int64)

    cfg.B = B
    cfg.IDXC = IDXC
    cfg.IDXC_SEG = IDXC_SEG
    # one gather per (segment, tile)
    cfg.G = NSEG * T
    cfg.PAIR_NB = B.copy()
    cfg.PAIR_NCH = (B + P - 1) // P
    cfg.JOBS = {}
    jcol = np.zeros((NSEG, T), dtype=np.int64)  # dest col base per bucket
    dcols = 0
    for s in range(NSEG):
        for t in range(T):
            nch = int(cfg.PAIR_NCH[s, t])
            cfg.JOBS[(s, t)] = [(0, c) for c in range(nch)]
            jcol[s, t] = dcols
            dcols += nch
    cfg.DCOLS = dcols
    cfg.CHUNKS = dcols  # (column count of the dest table)
    cfg.MAXCHP = int(cfg.PAIR_NCH.max())
    cfg.MAXJOBS = max(len(j) for j in cfg.JOBS.values())

    # bucket offsets in slots, (s, t) order (pair-contiguous)
    off = np.zeros((NSEG, T), dtype=np.int64)
    acc = 0
    for s in range(NSEG):
        for t in range(T):
            off[s, t] = acc
            acc += B[s, t]

    idx16 = np.full((M, 16, IDXC), -1, dtype=np.int16)
    dest = np.full((M, P, dcols), -1.0, dtype=np.float32)
    cnts = np.zeros((M, cfg.G), dtype=np.int32)
    for m in range(M):
        r, c, t, s = per_core[m]
        # slot index within each (s,t) bucket, in sorted order
        key = s * T + t
        change = np.flatnonzero(np.diff(key)) + 1
        starts = np.concatenate(([0], change))
        lens = np.diff(np.concatenate((starts, [len(key)])))
        within = np.arange(len(key)) - np.repeat(starts, lens)
        slot = off[s, t] + within
        idx16[m, slot % 16, slot // 16] = (c - seg_starts[s]).astype(np.int16)
        # dest col: bucket-local slot -> (partition, job column)
        jidx = jcol[s, t] + within // P
        dest[m, within % P, jidx] = (r - t * P).astype(np.float32)
        # per-core gather count: own (16-rounded) edge count
        cnts[m] = cnt16[m].reshape(-1)

    idx128 = np.tile(idx16, (1, 8, 1))  # replicate to 128 partitions
    return idx128, dest, cnts


def prep_host(cfg: Cfg, inputs):
    """Build per-core input maps (shared program, per-core data)."""
    x = np.asarray(inputs["x"], dtype=np.float32)
    N = cfg.N
    xT = np.zeros((D, cfg.NPAD), dtype=ml_dtypes.bfloat16)
    xT[:, :N] = x.T.astype(ml_dtypes.bfloat16)

    idx128, dest, cnts = _prep_edges(cfg, inputs["row"], inputs["col"])

    def bcast(v):
        return np.tile(np.asarray(v, np.float32)[None, :], (P, 1))

    att = np.asarray(inputs["att"], np.float32)
    shared = {
        "xT": xT,
        "W0T": np.asarray(inputs["W0"], np.float32).T.astype(ml_dtypes.bfloat16).copy(),
        "W1T": np.asarray(inputs["W1"], np.float32).T.astype(ml_dtypes.bfloat16).copy(),
        "b0c": np.asarray(inputs["b0"], np.float32).reshape(P, 1).copy(),
        "b1c": np.asarray(inputs["b1"], np.float32).reshape(P, 1).copy(),
        "att1b": bcast(att[:D]).copy(),
        "att2c": att[D:].astype(ml_dtypes.bfloat16).reshape(P, 1).copy(),
        "ones_r": np.ones((1, P), dtype=ml_dtypes.bfloat16),
        "ident_bf": np.eye(P, dtype=ml_dtypes.bfloat16),
        "ident_f": np.eye(P, dtype=np.float32),
        "iota_c": np.tile(np.arange(P, dtype=np.float32).astype(ml_dtypes.bfloat16)[None, :], (P, 1)),
        "scale0b": bcast(inputs["scale0"]).copy(),
        "scale1b": bcast(inputs["scale1"]).copy(),
        "off0b": bcast(inputs["offset0"]).copy(),
        "off1b": bcast(inputs["offset1"]).copy(),
    }
    in_maps = []
    for m in range(cfg.M):
        im = dict(shared)
        im["x_ownT"] = np.ascontiguousarray(
            xT[:, m * cfg.NC : m * cfg.NC + cfg.NC_PAD]
        )
        im["idx"] = np.ascontiguousarray(idx128[m])
        im["dest"] = dest[m].astype(ml_dtypes.bfloat16).copy()
        im["cnts"] = cnts[m : m + 1].copy()
        in_maps.append(im)
    return in_maps


def build(nc: bass.Bass, cfg: Cfg, simple_affine: bool):
    """Emit the full program, pipelined per source segment."""
    T, NSEG, SB = cfg.T, cfg.NSEG, cfg.SB
    B = cfg.B
    IDXC_SEG_MAX = int(max(cfg.IDXC_SEG))

    io = {}
    def inp(name, shape, dt):
        io[name] = nc.dram_tensor(name, list(shape), dt, kind="ExternalInput").ap()

    inp("xT", (D, cfg.NPAD), BF16)
    inp("x_ownT", (D, cfg.NC_PAD), BF16)
    inp("W0T", (D, D), BF16)
    inp("W1T", (D, D), BF16)
    inp("b0c", (P, 1), F32)
    inp("b1c", (P, 1), F32)
    inp("att1b", (P, D), F32)
    inp("att2c", (P, 1), BF16)
    inp("ones_r", (1, P), BF16)
    inp("ident_bf", (P, P), BF16)
    inp("ident_f", (P, P), F32)
    inp("iota_c", (P, P), BF16)
    inp("scale0b", (P, D), F32)
    inp("scale1b", (P, D), F32)
    inp("off0b", (P, D), F32)
    inp("off1b", (P, D), F32)
    inp("idx", (P, cfg.IDXC), I16)
    inp("dest", (P, cfg.CHUNKS), BF16)
    inp("cnts", (1, cfg.G), I32)
    out_d = nc.dram_tensor("out", [cfg.NC_PAD, D], F32, kind="ExternalOutput").ap()
    payload = nc.dram_tensor("payload", [cfg.NPAD, 2 * D], BF16, kind="Internal").ap()

    with tile.TileContext(nc) as tc, ExitStack() as ctx:
        singles = ctx.enter_context(tc.tile_pool(name="singles", bufs=1))
        xpool = ctx.enter_context(tc.tile_pool(name="xpool", bufs=3))
        hpool = ctx.enter_context(tc.tile_pool(name="hpool", bufs=3))
        ppool = ctx.enter_context(tc.tile_pool(name="ppool", bufs=3, space="PSUM"))
        pacc = ctx.enter_context(tc.tile_pool(name="pacc", bufs=3, space="PSUM"))
        pepi = ctx.enter_context(tc.tile_pool(name="pepi", bufs=2, space="PSUM"))
        gpool = ctx.enter_context(tc.tile_pool(name="gpool", bufs=10))
        p01pool = ctx.enter_context(tc.tile_pool(name="p01pool", bufs=6))
        ipool = ctx.enter_context(tc.tile_pool(name="ipool", bufs=2))
        epool = ctx.enter_context(tc.tile_pool(name="epool", bufs=4))

        # ---- constants ----
        def load(name, shape, dt):
            t = singles.tile(list(shape), dt, name=f"sb_{name}")
            nc.sync.dma_start(out=t, in_=io[name])
            return t

        W0T_sb = load("W0T", (D, D), BF16)
        W1T_sb = load("W1T", (D, D), BF16)
        b0c_sb = load("b0c", (P, 1), F32)
        b1c_sb = load("b1c", (P, 1), F32)
        att1b_sb = load("att1b", (P, D), F32)
        att2c_sb = load("att2c", (P, 1), BF16)
        ones_sb = load("ones_r", (1, P), BF16)
        dest_sb = load("dest", (P, cfg.CHUNKS), BF16)
        cnts_sb = load("cnts", (1, cfg.G), I32)
        if not simple_affine:
            scale0_sb = load("scale0b", (P, D), F32)
            scale1_sb = load("scale1b", (P, D), F32)
            off0_sb = load("off0b", (P, D), F32)
            off1_sb = load("off1b", (P, D), F32)
            off01_sb = singles.tile([P, D], F32, name="off01")
            nc.vector.tensor_tensor(
                out=off01_sb, in0=off0_sb, in1=off1_sb, op=mybir.AluOpType.add
            )

        ident_bf = load("ident_bf", (P, P), BF16)
        ident_f = load("ident_f", (P, P), F32)
        iota_bf = load("iota_c", (P, P), BF16)
        nc.gpsimd.load_library(library_config.mlp)

        nbreg = nc.alloc_register(mybir.EngineType.Pool, name="nbreg")

        alpha_sb = singles.tile([P, 1], F32, name="alpha_sb")
        nc.vector.memset(alpha_sb, 0.2)
        eps_sb = singles.tile([P, 1], F32, name="eps_sb")
        nc.vector.memset(eps_sb, 1e-9)
        agg_sb = singles.tile([P, T * 2 * D], F32, name="agg_sb")

        # ---- phase 1b superblock: h1 -> payload [U|V] for block i ----
        def emit_sblock(i):
            xb = xpool.tile([P, SB], BF16, name="xb", tag="xb")
            nc.sync.dma_start(out=xb, in_=io["xT"][:, i * SB : (i + 1) * SB])
            ps1 = ppool.tile([P, SB], F32, name="ps1", tag="ps")
            nc.tensor.matmul(out=ps1, lhsT=W1T_sb, rhs=xb, start=True, stop=True)
            h1T = hpool.tile([P, SB], BF16, name="h1T", tag="h1T")
            nc.scalar.activation(
                out=h1T, in_=ps1, func=mybir.ActivationFunctionType.Relu,
                bias=b1c_sb, scale=1.0,
            )
            psw = ppool.tile([1, SB], F32, name="psw", tag="ps")
            nc.tensor.matmul(out=psw, lhsT=att2c_sb, rhs=h1T, start=True, stop=True)
            wrow = hpool.tile([1, SB], BF16, name="wrow", tag="wrow")
            nc.scalar.activation(
                out=wrow, in_=psw, func=mybir.ActivationFunctionType.Prelu,
                scale=1.0, alpha=alpha_sb[0:1, :],
            )
            pswb = ppool.tile([P, SB], F32, name="pswb", tag="ps")
            nc.tensor.matmul(out=pswb, lhsT=ones_sb, rhs=wrow, start=True, stop=True)
            vT = hpool.tile([P, SB], BF16, name="vT", tag="vT")
            nc.vector.tensor_tensor(
                out=vT, in0=h1T, in1=pswb, op=mybir.AluOpType.mult
            )
            psuv = ppool.tile([P, 2 * SB], BF16, name="psuv", tag="ps")
            for j in range(SB // P):
                nc.tensor.transpose(
                    out=psuv[:, j * 256 : j * 256 + 128],
                    in_=h1T[:, j * P : (j + 1) * P], identity=ident_bf,
                )
                nc.tensor.transpose(
                    out=psuv[:, j * 256 + 128 : (j + 1) * 256],
                    in_=vT[:, j * P : (j + 1) * P], identity=ident_bf,
                )
            uv = hpool.tile([P, 2 * SB], BF16, name="uv", tag="uv")
            if i % 2 == 0:
                nc.scalar.copy(out=uv, in_=psuv)
            else:
                nc.vector.tensor_copy(out=uv, in_=psuv)
            nc.sync.dma_start(
                out=payload[i * SB : (i + 1) * SB, :].rearrange(
                    "(b p) e -> p b e", p=P
                ),
                in_=uv.rearrange("p (b e) -> p b e", e=2 * D),
            )

        # ---- epilogue for dest tile t: h0 recompute + norms + output ----
        def emit_epilogue(t):
            xo = xpool.tile([P, P], BF16, name="xo", tag="xo")
            nc.sync.dma_start(out=xo, in_=io["x_ownT"][:, t * P : (t + 1) * P])
            ps0 = pepi.tile([P, P], F32, name="ps0", tag="ps0")
            nc.tensor.matmul(out=ps0, lhsT=W0T_sb, rhs=xo, start=True, stop=True)
            h0T = epool.tile([P, P], F32, name="h0T", tag="h0T")
            nc.scalar.activation(
                out=h0T, in_=ps0, func=mybir.ActivationFunctionType.Relu,
                bias=b0c_sb, scale=1.0,
            )
            psT = pepi.tile([P, P], F32, name="psT", tag="ps0")
            nc.tensor.transpose(out=psT, in_=h0T, identity=ident_f)
            h0_t = epool.tile([P, P], F32, name="h0_t", tag="h0_t")
            nc.vector.tensor_copy(out=h0_t, in_=psT)
            tmp = epool.tile([P, P], F32, name="tmp", tag="tmp")
            nc.vector.tensor_tensor(
                out=tmp, in0=h0_t, in1=att1b_sb, op=mybir.AluOpType.mult
            )
            z = epool.tile([P, 1], F32, name="z", tag="z")
            nc.vector.tensor_reduce(
                out=z, in_=tmp, axis=mybir.AxisListType.X, op=mybir.AluOpType.add
            )
            a_col = epool.tile([P, 1], F32, name="a_col", tag="z")
            nc.scalar.activation(
                out=a_col, in_=z,
                func=mybir.ActivationFunctionType.Prelu, scale=1.0, alpha=alpha_sb,
            )
            agg_t = agg_sb[:, t * 2 * D : (t + 1) * 2 * D]
            bagg = epool.tile([P, D], F32, name="bagg", tag="bagg")
            nc.vector.tensor_scalar(
                bagg, agg_t[:, :D], a_col, None, mybir.AluOpType.mult,
            )
            nc.vector.tensor_tensor(
                out=bagg, in0=bagg, in1=agg_t[:, D:], op=mybir.AluOpType.add
            )

            def norm_stats(src, tag):
                st = epool.tile([P, 6], F32, name=f"st{tag}", tag=f"st{tag}")
                nc.vector.bn_stats(out=st, in_=src)
                mv = epool.tile([P, 2], F32, name=f"mv{tag}", tag=f"mv{tag}")
                nc.vector.bn_aggr(out=mv, in_=st)
                rstd = epool.tile([P, 1], F32, name=f"rs{tag}", tag=f"rs{tag}")
                nc.scalar.activation(
                    out=rstd, in_=mv[:, 1:2],
                    func=mybir.ActivationFunctionType.Sqrt, bias=eps_sb,
                )
                nc.vector.reciprocal(out=rstd, in_=rstd)
                return mv[:, 0:1], rstd

            m0, r0 = norm_stats(h0_t, "0")
            m1, r1 = norm_stats(bagg, "1")
            na = epool.tile([P, D], F32, name="na", tag="na")
            nc.vector.tensor_scalar(
                na, h0_t, m0, r0, mybir.AluOpType.subtract, mybir.AluOpType.mult
            )
            nb_ = epool.tile([P, D], F32, name="nb_", tag="nb_")
            nc.vector.tensor_scalar(
                nb_, bagg, m1, r1, mybir.AluOpType.subtract, mybir.AluOpType.mult
            )
            ot = epool.tile([P, D], F32, name="ot", tag="ot")
            if simple_affine:
                nc.vector.tensor_tensor(
                    out=ot, in0=na, in1=nb_, op=mybir.AluOpType.add
                )
            else:
                nc.vector.tensor_tensor(
                    out=na, in0=na, in1=scale0_sb, op=mybir.AluOpType.mult
                )
                nc.vector.tensor_tensor(
                    out=nb_, in0=nb_, in1=scale1_sb, op=mybir.AluOpType.mult
                )
                nc.vector.tensor_tensor(
                    out=na, in0=na, in1=nb_, op=mybir.AluOpType.add
                )
                nc.vector.tensor_tensor(
                    out=ot, in0=na, in1=off01_sb, op=mybir.AluOpType.add
                )
            nc.sync.dma_start(out=out_d[t * P : (t + 1) * P, :], in_=ot)

        # ---- preamble: segment 0 payload + its idx table ----
        idx_tiles = {}
        o16_seg = [0]
        for s in range(NSEG):
            o16_seg.append(o16_seg[-1] + int(cfg.IDXC_SEG[s]))

        def emit_idx_load(s):
            it = ipool.tile([P, IDXC_SEG_MAX], I16, name="idxseg", tag="idxseg")
            w = int(cfg.IDXC_SEG[s])
            nc.sync.dma_start(
                out=it[:, :w], in_=io["idx"][:, o16_seg[s] : o16_seg[s] + w]
            )
            idx_tiles[s] = it

        emit_idx_load(0)
        # pre-zero the rotating gather buffers: slots past a core's own
        # count are never written by the gather, and 0 x NaN garbage
        # would poison the PSUM accumulation
        for _ in range(10):
            gb0 = gpool.tile([P, cfg.MAXCHP * 2 * D], BF16, name="gb", tag="gb")
            nc.vector.memset(gb0, 0.0)
        sb_base = [0]
        for s in range(NSEG):
            sb_base.append(sb_base[-1] + cfg.SBSEG[s])
        for k in range(cfg.SBSEG[0]):
            emit_sblock(k)
        tc.strict_bb_all_engine_barrier()

        # ---- segment loop ----
        gcol = 0  # running chunk column
        for s in range(NSEG):
            sb_next = sb_base[s + 1]  # first superblock of next segment
            sb_quota = cfg.SBSEG[s + 1] if s < NSEG - 1 else 0
            sb_emitted = 0
            seg_base = int(cfg.SEG_STARTS[s])
            seg_len = int(cfg.SEG_SIZES[s])
            o16 = 0  # idx column offset within segment tile
            idx_sb = idx_tiles[s]
            for t in range(T):
                nb = int(cfg.PAIR_NB[s][t])
                nch = int(cfg.PAIR_NCH[s][t])
                jobs = cfg.JOBS[(s, t)]
                njobs = len(jobs)
                g = s * T + t
                if nb > 0:
                    nc.gpsimd.reg_load(nbreg, cnts_sb[0:1, g : g + 1])
                    gb = gpool.tile([P, cfg.MAXCHP * 2 * D], BF16, name="gb", tag="gb")
                    nc.gpsimd.dma_gather(
                        out_ap=gb[:, : nch * 256].rearrange(
                            "p (c e) -> p c e", e=256
                        ),
                        in_ap=payload[seg_base : seg_base + seg_len, :],
                        idxs_ap=idx_sb[:, o16 : o16 + nb // 16],
                        num_idxs=nb,
                        num_idxs_reg=nbreg,
                        elem_size=256,
                        single_packet=False,
                        queue_num=g % 4,
                    )
                    o16 += nb // 16
                    # batched one-hot: p01[p, j*128 + f] = (iota[f] == dest[p, gcol+j])
                    p01 = p01pool.tile(
                        [P, cfg.MAXJOBS * P], BF16, name="p01", tag="p01"
                    )
                    nc.vector.tensor_tensor(
                        out=p01[:, : njobs * P].rearrange("p (c f) -> p c f", f=P),
                        in0=iota_bf.unsqueeze(1).broadcast_to([P, njobs, P]),
                        in1=dest_sb[:, gcol : gcol + njobs]
                        .unsqueeze(2)
                        .broadcast_to([P, njobs, P]),
                        op=mybir.AluOpType.is_equal,
                    )
                    ps_acc = pacc.tile(
                        [P, 2 * D], F32, name="ps_acc", tag="ps_acc"
                    )
                    for k, (_, chunk) in enumerate(jobs):
                        nc.tensor.matmul(
                            out=ps_acc,
                            lhsT=p01[:, k * P : (k + 1) * P],
                            rhs=gb[:, chunk * 256 : (chunk + 1) * 256],
                            start=(k == 0),
                            stop=(k == njobs - 1),
                        )
                    agg_t = agg_sb[:, t * 2 * D : (t + 1) * 2 * D]
                    if s == 0:
                        nc.vector.tensor_copy(out=agg_t, in_=ps_acc)
                    else:
                        nc.vector.tensor_tensor(
                            out=agg_t, in0=agg_t, in1=ps_acc,
                            op=mybir.AluOpType.add,
                        )
                    gcol += njobs
                elif s == 0:
                    nc.vector.memset(
                        agg_sb[:, t * 2 * D : (t + 1) * 2 * D], 0.0
                    )

                if s < NSEG - 1:
                    # interleave next segment's payload build + idx load,
                    # paced evenly across this segment's tile loop
                    if t == 1:
                        emit_idx_load(s + 1)
                    while sb_emitted < min(sb_quota, (t + 1) * sb_quota // T):
                        emit_sblock(sb_next + sb_emitted)
                        sb_emitted += 1
                else:
                    emit_epilogue(t)
            if s < NSEG - 1:
                while sb_emitted < sb_quota:
                    emit_sblock(sb_next + sb_emitted)
                    sb_emitted += 1
                tc.strict_bb_all_engine_barrier()
    return io


def make_program(cfg: Cfg, inputs):
    in_maps = prep_host(cfg, inputs)
    simple_affine = (
        np.all(np.asarray(inputs["scale0"]) == 1.0)
        and np.all(np.asarray(inputs["scale1"]) == 1.0)
        and np.all(np.asarray(inputs["offset0"]) == 0.0)
        and np.all(np.asarray(inputs["offset1"]) == 0.0)
    )
    nc = bacc.Bacc(
        "TRN2", target_bir_lowering=False, debug=False, enable_asserts=False,
        num_devices=cfg.M, num_swdge_queues=4,
    )
    build(nc, cfg, bool(simple_affine))
    nc.compile()
    return nc, in_maps


_cache = {}


def kernel(**inputs) -> np.ndarray:
    x = np.asarray(inputs["x"])
    n_nodes = x.shape[0]
    n_cores = 8
    key = hashlib.sha1(
        np.asarray(inputs["row"]).tobytes() + np.asarray(inputs["col"]).tobytes()
    ).hexdigest() + f"_{n_nodes}"
    if key in _cache:
        cfg, nc, _ = _cache[key]
        in_maps = prep_host(cfg, inputs)
    else:
        cfg = Cfg(n_nodes, n_cores)
        nc, in_maps = make_program(cfg, inputs)
        _cache[key] = (cfg, nc, in_maps)

    res = bass_utils.run_bass_kernel_spmd(
        nc, in_maps, core_ids=list(range(n_cores))
    )
    out = np.concatenate(
        [res.results[m]["out"][: cfg.NC] for m in range(n_cores)], axis=0
    )
    return out.astype(np.float32)


# revision 48
# speedup vs baseline: 1.0338x; 1.0338x over previous
"""Trainium2 Bass kernel for nn_AttentionAggregator (GNN message passing).

out = norm(h0)*scale0+offset0 + norm(agg)*scale1+offset1
  h0 = relu(x@W0.T + b0); h1 = relu(x@W1.T + b1)
  a_self = lrelu(h0@att[:d]); a_neigh = lrelu(h1@att[d:])
  agg[i] = sum_{(i,j) in E} (a_self[i]+a_neigh[j]) * h1[j]

Strategy (8 cores, SPMD, no collectives):
  - nodes (rows of output) sharded across cores; edges partitioned by dest row
  - every core recomputes full h1 and writes a node-major payload
    row[j] = [h1[j] | a_neigh[j]*h1[j]] (256 bf16 = 512 B) to its HBM
  - payload build is pipelined per ascending source segment with the
    per-dest-tile dma_gather + one-hot PE segment-sum; gathers alternate
    4 SWDGE queues to overlap descriptor gen with ring drain; per-core
    exact counts via reg_load (trimmed tail idx pads MUST be -1)
  - agg partials accumulate in SBUF f32 across segments; the epilogue
    recomputes h0 per tile and fuses both norms
"""

import hashlib
from contextlib import ExitStack

import numpy as np
import ml_dtypes

import concourse.bass as bass
import concourse.bacc as bacc
import concourse.tile as tile
import concourse.mybir as mybir
from concourse import bass_utils
from concourse import library_config

BF16 = mybir.dt.bfloat16
F32 = mybir.dt.float32
I16 = mybir.dt.int16
I32 = mybir.dt.int32

D = 128  # feature dim (both in and out)
P = 128  # partitions


class Cfg:
    def __init__(self, n_nodes, n_cores):
        assert n_nodes % n_cores == 0
        self.N = n_nodes
        self.M = n_cores
        self.NC = n_nodes // n_cores          # dest rows per core
        self.T = (self.NC + P - 1) // P       # dest tiles per core
        self.NC_PAD = self.T * P
        self.SB = 512                         # phase-1 node superblock
        self.NB = (n_nodes + self.SB - 1) // self.SB
        self.NPAD = self.NB * self.SB
        # ascending source segments (rows); each <= 32768 (int16 idx range),
        # each a multiple of SB. A tiny first segment lets the gathers start
        # almost immediately while later payload builds hide under them.
        sizes = []
        rem = self.NPAD
        for want in (2048, 8192):
            if rem > want * 2:
                sizes.append(want)
                rem -= want
        # near-equal tail segments (512-mult), no runt remainder
        ntail = (rem + 16384 - 1) // 16384
        base = (rem // ntail) // self.SB * self.SB
        extra = (rem - ntail * base) // self.SB
        for i in range(ntail):
            sizes.append(base + self.SB * (1 if i < extra else 0))
        assert sum(sizes) == self.NPAD
        self.SEG_SIZES = sizes
        self.SEG_STARTS = np.concatenate(([0], np.cumsum(sizes))).astype(np.int64)
        self.NSEG = len(sizes)
        self.SBSEG = [sz // self.SB for sz in sizes]  # superblocks per segment
        # filled by prep():
        self.B = None        # [NSEG][T] bucket layout sizes (16-mult, shared)
        self.CHUNKS = None   # total chunks
        self.IDXC = None     # total idx columns (sum B / 16)
        self.IDXC_SEG = None # [NSEG] idx columns per segment
        self.G = None        # NSEG*T gather count


def _prep_edges(cfg: Cfg, row, col):
    """Sort edges per core into (segment, tile) buckets; build tables."""
    M, NC, T, NSEG = cfg.M, cfg.NC, cfg.T, cfg.NSEG
    row = np.asarray(row).astype(np.int64)
    col = np.asarray(col).astype(np.int64)
    seg_starts = cfg.SEG_STARTS

    per_core = []
    counts = np.zeros((M, NSEG, T), dtype=np.int64)
    for m in range(M):
        mask = (row >= m * NC) & (row < (m + 1) * NC)
        r = row[mask] - m * NC
        c = col[mask]
        t = r >> 7
        s = np.searchsorted(seg_starts, c, side="right") - 1
        order = np.lexsort((c, t, s))
        r, c, t, s = r[order], c[order], t[order], s[order]
        np.add.at(counts[m], (s, t), 1)
        per_core.append((r, c, t, s))

    cnt16 = ((counts + 15) // 16) * 16            # per-core 16-mult counts
    B = cnt16.max(axis=0)                          # [NSEG, T] shared layout
    CH = (B + P - 1) // P                          # chunks per bucket
    CHUNKS = int(CH.sum())
    TOT = int(B.sum())
    IDXC = TOT // 16
    IDXC_SEG = (B.sum(axis=1) // 16).astype(np.int64)

    cfg.B = B
    cfg.IDXC = IDXC
    cfg.IDXC_SEG = IDXC_SEG
    # one gather per (segment, tile)
    cfg.G = NSEG * T
    cfg.PAIR_NB = B.copy()
    cfg.PAIR_NCH = (B + P - 1) // P
    cfg.JOBS = {}
    jcol = np.zeros((NSEG, T), dtype=np.int64)  # dest col base per bucket
    dcols = 0
    for s in range(NSEG):
        for t in range(T):
            nch = int(cfg.PAIR_NCH[s, t])
            cfg.JOBS[(s, t)] = [(0, c) for c in range(nch)]
            jcol[s, t] = dcols
            dcols += nch
    cfg.DCOLS = dcols
    cfg.CHUNKS = dcols  # (column count of the dest table)
    cfg.MAXCHP = int(cfg.PAIR_NCH.max())
    cfg.MAXJOBS = max(len(j) for j in cfg.JOBS.values())

    # bucket offsets in slots, (s, t) order (pair-contiguous)
    off = np.zeros((NSEG, T), dtype=np.int64)
    acc = 0
    for s in range(NSEG):
        for t in range(T):
            off[s, t] = acc
            acc += B[s, t]

    idx16 = np.full((M, 16, IDXC), -1, dtype=np.int16)
    dest = np.full((M, P, dcols), -1.0, dtype=np.float32)
    cnts = np.zeros((M, cfg.G), dtype=np.int32)
    for m in range(M):
        r, c, t, s = per_core[m]
        # slot index within each (s,t) bucket, in sorted order
        key = s * T + t
        change = np.flatnonzero(np.diff(key)) + 1
        starts = np.concatenate(([0], change))
        lens = np.diff(np.concatenate((starts, [len(key)])))
        within = np.arange(len(key)) - np.repeat(starts, lens)
        slot = off[s, t] + within
        idx16[m, slot % 16, slot // 16] = (c - seg_starts[s]).astype(np.int16)
        # dest col: bucket-local slot -> (partition, job column)
        jidx = jcol[s, t] + within // P
        dest[m, within % P, jidx] = (r - t * P).astype(np.float32)
        # per-core gather count: own (16-rounded) edge count
        cnts[m] = cnt16[m].reshape(-1)

    idx128 = np.tile(idx16, (1, 8, 1))  # replicate to 128 partitions
    return idx128, dest, cnts


def prep_host(cfg: Cfg, inputs):
    """Build per-core input maps (shared program, per-core data)."""
    x = np.asarray(inputs["x"], dtype=np.float32)
    N = cfg.N
    xT = np.zeros((D, cfg.NPAD), dtype=ml_dtypes.bfloat16)
    xT[:, :N] = x.T.astype(ml_dtypes.bfloat16)

    idx128, dest, cnts = _prep_edges(cfg, inputs["row"], inputs["col"])

    def bcast(v):
        return np.tile(np.asarray(v, np.float32)[None, :], (P, 1))

    att = np.asarray(inputs["att"], np.float32)
    shared = {
        "xT": xT,
        "W0T": np.asarray(inputs["W0"], np.float32).T.astype(ml_dtypes.bfloat16).copy(),
        "W1T": np.asarray(inputs["W1"], np.float32).T.astype(ml_dtypes.bfloat16).copy(),
        "b0c": np.asarray(inputs["b0"], np.float32).reshape(P, 1).copy(),
        "b1c": np.asarray(inputs["b1"], np.float32).reshape(P, 1).copy(),
        "att1b": bcast(att[:D]).copy(),
        "att2c": att[D:].astype(ml_dtypes.bfloat16).reshape(P, 1).copy(),
        "ones_r": np.ones((1, P), dtype=ml_dtypes.bfloat16),
        "ident_bf": np.eye(P, dtype=ml_dtypes.bfloat16),
        "ident_f": np.eye(P, dtype=np.float32),
        "iota_c": np.tile(np.arange(P, dtype=np.float32).astype(ml_dtypes.bfloat16)[None, :], (P, 1)),
        "scale0b": bcast(inputs["scale0"]).copy(),
        "scale1b": bcast(inputs["scale1"]).copy(),
        "off0b": bcast(inputs["offset0"]).copy(),
        "off1b": bcast(inputs["offset1"]).copy(),
    }
    in_maps = []
    for m in range(cfg.M):
        im = dict(shared)
        im["x_ownT"] = np.ascontiguousarray(
            xT[:, m * cfg.NC : m * cfg.NC + cfg.NC_PAD]
        )
        im["idx"] = np.ascontiguousarray(idx128[m])
        im["dest"] = dest[m].astype(ml_dtypes.bfloat16).copy()
        im["cnts"] = cnts[m : m + 1].copy()
        in_maps.append(im)
    return in_maps


def build(nc: bass.Bass, cfg: Cfg, simple_affine: bool):
    """Emit the full program, pipelined per source segment."""
    T, NSEG, SB = cfg.T, cfg.NSEG, cfg.SB
    B = cfg.B
    IDXC_SEG_MAX = int(max(cfg.IDXC_SEG))

    io = {}
    def inp(name, shape, dt):
        io[name] = nc.dram_tensor(name, list(shape), dt, kind="ExternalInput").ap()

    inp("xT", (D, cfg.NPAD), BF16)
    inp("x_ownT", (D, cfg.NC_PAD), BF16)
    inp("W0T", (D, D), BF16)
    inp("W1T", (D, D), BF16)
    inp("b0c", (P, 1), F32)
    inp("b1c", (P, 1), F32)
    inp("att1b", (P, D), F32)
    inp("att2c", (P, 1), BF16)
    inp("ones_r", (1, P), BF16)
    inp("ident_bf", (P, P), BF16)
    inp("ident_f", (P, P), F32)
    inp("iota_c", (P, P), BF16)
    inp("scale0b", (P, D), F32)
    inp("scale1b", (P, D), F32)
    inp("off0b", (P, D), F32)
    inp("off1b", (P, D), F32)
    inp("idx", (P, cfg.IDXC), I16)
    inp("dest", (P, cfg.CHUNKS), BF16)
    inp("cnts", (1, cfg.G), I32)
    out_d = nc.dram_tensor("out", [cfg.NC_PAD, D], F32, kind="ExternalOutput").ap()
    payload = nc.dram_tensor("payload", [cfg.NPAD, 2 * D], BF16, kind="Internal").ap()

    with tile.TileContext(nc) as tc, ExitStack() as ctx:
        singles = ctx.enter_context(tc.tile_pool(name="singles", bufs=1))
        xpool = ctx.enter_context(tc.tile_pool(name="xpool", bufs=3))
        hpool = ctx.enter_context(tc.tile_pool(name="hpool", bufs=3))
        ppool = ctx.enter_context(tc.tile_pool(name="ppool", bufs=3, space="PSUM"))
        pacc = ctx.enter_context(tc.tile_pool(name="pacc", bufs=3, space="PSUM"))
        pepi = ctx.enter_context(tc.tile_pool(name="pepi", bufs=2, space="PSUM"))
        gpool = ctx.enter_context(tc.tile_pool(name="gpool", bufs=10))
        p01pool = ctx.enter_context(tc.tile_pool(name="p01pool", bufs=6))
        ipool = ctx.enter_context(tc.tile_pool(name="ipool", bufs=2))
        epool = ctx.enter_context(tc.tile_pool(name="epool", bufs=4))

        # ---- constants ----
        def load(name, shape, dt):
            t = singles.tile(list(shape), dt, name=f"sb_{name}")
            nc.sync.dma_start(out=t, in_=io[name])
            return t

        W0T_sb = load("W0T", (D, D), BF16)
        W1T_sb = load("W1T", (D, D), BF16)
        b0c_sb = load("b0c", (P, 1), F32)
        b1c_sb = load("b1c", (P, 1), F32)
        att1b_sb = load("att1b", (P, D), F32)
        att2c_sb = load("att2c", (P, 1), BF16)
        ones_sb = load("ones_r", (1, P), BF16)
        dest_sb = load("dest", (P, cfg.CHUNKS), BF16)
        cnts_sb = load("cnts", (1, cfg.G), I32)
        if not simple_affine:
            scale0_sb = load("scale0b", (P, D), F32)
            scale1_sb = load("scale1b", (P, D), F32)
            off0_sb = load("off0b", (P, D), F32)
            off1_sb = load("off1b", (P, D), F32)
            off01_sb = singles.tile([P, D], F32, name="off01")
            nc.vector.tensor_tensor(
                out=off01_sb, in0=off0_sb, in1=off1_sb, op=mybir.AluOpType.add
            )

        ident_bf = load("ident_bf", (P, P), BF16)
        ident_f = load("ident_f", (P, P), F32)
        iota_bf = load("iota_c", (P, P), BF16)
        nc.gpsimd.load_library(library_config.mlp)

        nbreg = nc.alloc_register(mybir.EngineType.Pool, name="nbreg")

        alpha_sb = singles.tile([P, 1], F32, name="alpha_sb")
        nc.vector.memset(alpha_sb, 0.2)
        eps_sb = singles.tile([P, 1], F32, name="eps_sb")
        nc.vector.memset(eps_sb, 1e-9)
        agg_sb = singles.tile([P, T * 2 * D], F32, name="agg_sb")

        # ---- phase 1b superblock: h1 -> payload [U|V] for block i ----
        def emit_sblock(i):
            xb = xpool.tile([P, SB], BF16, name="xb", tag="xb")
            nc.sync.dma_start(out=xb, in_=io["xT"][:, i * SB : (i + 1) * SB])
            ps1 = ppool.tile([P, SB], F32, name="ps1", tag="ps")
            nc.tensor.matmul(out=ps1, lhsT=W1T_sb, rhs=xb, start=True, stop=True)
            h1T = hpool.tile([P, SB], BF16, name="h1T", tag="h1T")
            nc.scalar.activation(
                out=h1T, in_=ps1, func=mybir.ActivationFunctionType.Relu,
                bias=b1c_sb, scale=1.0,
            )
            psw = ppool.tile([1, SB], F32, name="psw", tag="ps")
            nc.tensor.matmul(out=psw, lhsT=att2c_sb, rhs=h1T, start=True, stop=True)
            wrow = hpool.tile([1, SB], BF16, name="wrow", tag="wrow")
            nc.scalar.activation(
                out=wrow, in_=psw, func=mybir.ActivationFunctionType.Prelu,
                scale=1.0, alpha=alpha_sb[0:1, :],
            )
            pswb = ppool.tile([P, SB], F32, name="pswb", tag="ps")
            nc.tensor.matmul(out=pswb, lhsT=ones_sb, rhs=wrow, start=True, stop=True)
            vT = hpool.tile([P, SB], BF16, name="vT", tag="vT")
            nc.vector.tensor_tensor(
                out=vT, in0=h1T, in1=pswb, op=mybir.AluOpType.mult
            )
            psuv = ppool.tile([P, 2 * SB], BF16, name="psuv", tag="ps")
            for j in range(SB // P):
                nc.tensor.transpose(
                    out=psuv[:, j * 256 : j * 256 + 128],
                    in_=h1T[:, j * P : (j + 1) * P], identity=ident_bf,
                )
                nc.tensor.transpose(
                    out=psuv[:, j * 256 + 128 : (j + 1) * 256],
                    in_=vT[:, j * P : (j + 1) * P], identity=ident_bf,
                )
            uv = hpool.tile([P, 2 * SB], BF16, name="uv", tag="uv")
            if i % 2 == 0:
                nc.scalar.copy(out=uv, in_=psuv)
            else:
                nc.vector.tensor_copy(out=uv, in_=psuv)
            nc.sync.dma_start(
                out=payload[i * SB : (i + 1) * SB, :].rearrange(
                    "(b p) e -> p b e", p=P
                ),
                in_=uv.rearrange("p (b e) -> p b e", e=2 * D),
            )

        # ---- epilogue for dest tile t: h0 recompute + norms + output ----
        def emit_epilogue(t):
            xo = xpool.tile([P, P], BF16, name="xo", tag="xo")
            nc.sync.dma_start(out=xo, in_=io["x_ownT"][:, t * P : (t + 1) * P])
            ps0 = pepi.tile([P, P], F32, name="ps0", tag="ps0")
            nc.tensor.matmul(out=ps0, lhsT=W0T_sb, rhs=xo, start=True, stop=True)
            h0T = epool.tile([P, P], F32, name="h0T", tag="h0T")
            nc.scalar.activation(
                out=h0T, in_=ps0, func=mybir.ActivationFunctionType.Relu,
                bias=b0c_sb, scale=1.0,
            )
            psT = pepi.tile([P, P], F32, name="psT", tag="ps0")
            nc.tensor.transpose(out=psT, in_=h0T, identity=ident_f)
            h0_t = epool.tile([P, P], F32, name="h0_t", tag="h0_t")
            nc.vector.tensor_copy(out=h0_t, in_=psT)
            tmp = epool.tile([P, P], F32, name="tmp", tag="tmp")
            nc.vector.tensor_tensor(
                out=tmp, in0=h0_t, in1=att1b_sb, op=mybir.AluOpType.mult
            )
            z = epool.tile([P, 1], F32, name="z", tag="z")
            nc.vector.tensor_reduce(
                out=z, in_=tmp, axis=mybir.AxisListType.X, op=mybir.AluOpType.add
            )
            a_col = epool.tile([P, 1], F32, name="a_col", tag="z")
            nc.scalar.activation(
                out=a_col, in_=z,
                func=mybir.ActivationFunctionType.Prelu, scale=1.0, alpha=alpha_sb,
            )
            agg_t = agg_sb[:, t * 2 * D : (t + 1) * 2 * D]
            bagg = epool.tile([P, D], F32, name="bagg", tag="bagg")
            nc.vector.tensor_scalar(
                bagg, agg_t[:, :D], a_col, None, mybir.AluOpType.mult,
            )
            nc.vector.tensor_tensor(
                out=bagg, in0=bagg, in1=agg_t[:, D:], op=mybir.AluOpType.add
            )

            def norm_stats(src, tag):
                st = epool.tile([P, 6], F32, name=f"st{tag}", tag=f"st{tag}")
                nc.vector.bn_stats(out=st, in_=src)
                mv = epool.tile([P, 2], F32, name=f"mv{tag}", tag=f"mv{tag}")
                nc.vector.bn_aggr(out=mv, in_=st)
                rstd = epool.tile([P, 1], F32, name=f"rs{tag}", tag=f"rs{tag}")
                nc.scalar.activation(
                    out=rstd, in_=mv[:, 1:2],
                    func=mybir.ActivationFunctionType.Sqrt, bias=eps_sb,
                )
                nc.vector.reciprocal(out=rstd, in_=rstd)
                return mv[:, 0:1], rstd

            m0, r0 = norm_stats(h0_t, "0")
            m1, r1 = norm_stats(bagg, "1")
            na = epool.tile([P, D], F32, name="na", tag="na")
            nc.vector.tensor_scalar(
                na, h0_t, m0, r0, mybir.AluOpType.subtract, mybir.AluOpType.mult
            )
            nb_ = epool.tile([P, D], F32, name="nb_", tag="nb_")
            nc.vector.tensor_scalar(
                nb_, bagg, m1, r1, mybir.AluOpType.subtract, mybir.AluOpType.mult
            )
            ot = epool.tile([P, D], F32, name="ot", tag="ot")
            if simple_affine:
                nc.vector.tensor_tensor(
                    out=ot, in0=na, in1=nb_, op=mybir.AluOpType.add
                )
            else:
                nc.vector.tensor_tensor(
                    out=na, in0=na, in1=scale0_sb, op=mybir.AluOpType.mult
                )
                nc.vector.tensor_tensor(
                    out=nb_, in0=nb_, in1=scale1_sb, op=mybir.AluOpType.mult
                )
                nc.vector.tensor_tensor(
                    out=na, in0=na, in1=nb_, op=mybir.AluOpType.add
                )
                nc.vector.tensor_tensor(
                    out=ot, in0=na, in1=off01_sb, op=mybir.AluOpType.add
                )
            nc.sync.dma_start(out=out_d[t * P : (t + 1) * P, :], in_=ot)

        # ---- preamble: segment 0 payload + its idx table ----
        idx_tiles = {}
        o16_seg = [0]
        for s in range(NSEG):
            o16_seg.append(o16_seg[-1] + int(cfg.IDXC_SEG[s]))

        def emit_idx_load(s):
            it = ipool.tile([P, IDXC_SEG_MAX], I16, name="idxseg", tag="idxseg")
            w = int(cfg.IDXC_SEG[s])
            nc.sync.dma_start(
                out=it[:, :w], in_=io["idx"][:, o16_seg[s] : o16_seg[s] + w]
            )
            idx_tiles[s] = it

        emit_idx_load(0)
        # pre-zero the rotating gather buffers: slots past a core's own
        # count are never written by the gather, and 0 x NaN garbage
        # would poison the PSUM accumulation
        for _ in range(10):
            gb0 = gpool.tile([P, cfg.MAXCHP * 2 * D], BF16, name="gb", tag="gb")
            nc.vector.memset(gb0, 0.0)
        sb_base = [0]
        for s in range(NSEG):
            sb_base.append(sb_base[-1] + cfg.SBSEG[s])
        for k in range(cfg.SBSEG[0]):
            emit_sblock(k)
        tc.strict_bb_all_engine_barrier()

        # ---- segment loop ----
        gcol = 0  # running chunk column
        for s in range(NSEG):
            sb_next = sb_base[s + 1]  # first superblock of next segment
            sb_quota = cfg.SBSEG[s + 1] if s < NSEG - 1 else 0
            sb_emitted = 0
            seg_base = int(cfg.SEG_STARTS[s])
            seg_len = int(cfg.SEG_SIZES[s])
            o16 = 0  # idx column offset within segment tile
            idx_sb = idx_tiles[s]
            for t in range(T):
                nb = int(cfg.PAIR_NB[s][t])
                nch = int(cfg.PAIR_NCH[s][t])
                jobs = cfg.JOBS[(s, t)]
                njobs = len(jobs)
                g = s * T + t
                if nb > 0:
                    nc.gpsimd.reg_load(nbreg, cnts_sb[0:1, g : g + 1])
                    gb = gpool.tile([P, cfg.MAXCHP * 2 * D], BF16, name="gb", tag="gb")
                    nc.gpsimd.dma_gather(
                        out_ap=gb[:, : nch * 256].rearrange(
                            "p (c e) -> p c e", e=256
                        ),
                        in_ap=payload[seg_base : seg_base + seg_len, :],
                        idxs_ap=idx_sb[:, o16 : o16 + nb // 16],
                        num_idxs=nb,
                        num_idxs_reg=nbreg,
                        elem_size=256,
                        single_packet=False,
                        queue_num=g % 4,
                    )
                    o16 += nb // 16
                    # batched one-hot: p01[p, j*128 + f] = (iota[f] == dest[p, gcol+j])
                    p01 = p01pool.tile(
                        [P, cfg.MAXJOBS * P], BF16, name="p01", tag="p01"
                    )
                    nc.vector.tensor_tensor(
                        out=p01[:, : njobs * P].rearrange("p (c f) -> p c f", f=P),
                        in0=iota_bf.unsqueeze(1).broadcast_to([P, njobs, P]),
                        in1=dest_sb[:, gcol : gcol + njobs]
                        .unsqueeze(2)
                        .broadcast_to([P, njobs, P]),
                        op=mybir.AluOpType.is_equal,
                    )
                    ps_acc = pacc.tile(
                        [P, 2 * D], F32, name="ps_acc", tag="ps_acc"
                    )
                    for k, (_, chunk) in enumerate(jobs):
                        nc.tensor.matmul(
                            out=ps_acc,
                            lhsT=p01[:, k * P : (k + 1) * P],
                            rhs=gb[:, chunk * 256 : (chunk + 1) * 256],
                            start=(k == 0),
                            stop=(k == njobs - 1),
                        )
                    agg_t = agg_sb[:, t * 2 * D : (t + 1) * 2 * D]
                    if s == 0:
                        nc.vector.tensor_copy(out=agg_t, in_=ps_acc)
                    else:
                        nc.vector.tensor_tensor(
                            out=agg_t, in0=agg_t, in1=ps_acc,
                            op=mybir.AluOpType.add,
                        )
                    gcol += njobs
                elif s == 0:
                    nc.vector.memset(
                        agg_sb[:, t * 2 * D : (t + 1) * 2 * D], 0.0
                    )

                if s < NSEG - 1:
                    # interleave next segment's payload build + idx load,
                    # paced evenly across this segment's tile loop
                    if t == 1:
                        emit_idx_load(s + 1)
                    while sb_emitted < min(sb_quota, (t + 1) * sb_quota // T):
                        emit_sblock(sb_next + sb_emitted)
                        sb_emitted += 1
                else:
                    emit_epilogue(t)
            if s < NSEG - 1:
                while sb_emitted < sb_quota:
                    emit_sblock(sb_next + sb_emitted)
                    sb_emitted += 1
                tc.strict_bb_all_engine_barrier()
    return io


def make_program(cfg: Cfg, inputs):
    in_maps = prep_host(cfg, inputs)
    simple_affine = (
        np.all(np.asarray(inputs["scale0"]) == 1.0)
        and np.all(np.asarray(inputs["scale1"]) == 1.0)
        and np.all(np.asarray(inputs["offset0"]) == 0.0)
        and np.all(np.asarray(inputs["offset1"]) == 0.0)
    )
    nc = bacc.Bacc(
        "TRN2", target_bir_lowering=False, debug=False, enable_asserts=False,
        num_devices=cfg.M, num_swdge_queues=4,
    )
    build(nc, cfg, bool(simple_affine))
    nc.compile()
    return nc, in_maps


_cache = {}


def kernel(**inputs) -> np.ndarray:
    x = np.asarray(inputs["x"])
    n_nodes = x.shape[0]
    n_cores = 8
    key = hashlib.sha1(
        np.asarray(inputs["row"]).tobytes() + np.asarray(inputs["col"]).tobytes()
    ).hexdigest() + f"_{n_nodes}"
    if key in _cache:
        cfg, nc, _ = _cache[key]
        in_maps = prep_host(cfg, inputs)
    else:
        cfg = Cfg(n_nodes, n_cores)
        nc, in_maps = make_program(cfg, inputs)
        _cache[key] = (cfg, nc, in_maps)

    res = bass_utils.run_bass_kernel_spmd(
        nc, in_maps, core_ids=list(range(n_cores))
    )
    out = np.concatenate(
        [res.results[m]["out"][: cfg.NC] for m in range(n_cores)], axis=0
    )
    return out.astype(np.float32)


# revision 53
# speedup vs baseline: 1.0758x; 1.0407x over previous
"""Trainium2 Bass kernel for nn_AttentionAggregator (GNN message passing).

out = norm(h0)*scale0+offset0 + norm(agg)*scale1+offset1
  h0 = relu(x@W0.T + b0); h1 = relu(x@W1.T + b1)
  a_self = lrelu(h0@att[:d]); a_neigh = lrelu(h1@att[d:])
  agg[i] = sum_{(i,j) in E} (a_self[i]+a_neigh[j]) * h1[j]

Strategy (8 cores, SPMD, no collectives):
  - nodes (rows of output) sharded across cores; edges partitioned by dest row
  - every core recomputes full h1 and writes a node-major payload
    row[j] = [h1[j] | a_neigh[j]*h1[j]] (256 bf16 = 512 B) to its HBM
  - payload build is pipelined per ascending source segment with the
    per-dest-tile dma_gather + one-hot PE segment-sum; gathers alternate
    4 SWDGE queues to overlap descriptor gen with ring drain; per-core
    exact counts via reg_load (trimmed tail idx pads MUST be -1)
  - agg partials accumulate in SBUF f32 across segments; the epilogue
    recomputes h0 per tile and fuses both norms
"""

import hashlib
from contextlib import ExitStack

import numpy as np
import ml_dtypes

import concourse.bass as bass
import concourse.bacc as bacc
import concourse.tile as tile
import concourse.mybir as mybir
from concourse import bass_utils
from concourse import library_config

BF16 = mybir.dt.bfloat16
F32 = mybir.dt.float32
I16 = mybir.dt.int16
I32 = mybir.dt.int32

D = 128  # feature dim (both in and out)
P = 128  # partitions


class Cfg:
    def __init__(self, n_nodes, n_cores):
        assert n_nodes % n_cores == 0
        self.N = n_nodes
        self.M = n_cores
        self.NC = n_nodes // n_cores          # dest rows per core
        self.T = (self.NC + P - 1) // P       # dest tiles per core
        self.NC_PAD = self.T * P
        self.SB = 512                         # phase-1 node superblock
        self.NB = (n_nodes + self.SB - 1) // self.SB
        self.NPAD = self.NB * self.SB
        # ascending source segments (rows); each <= 32768 (int16 idx range),
        # each a multiple of SB. A tiny first segment lets the gathers start
        # almost immediately while later payload builds hide under them.
        sizes = []
        rem = self.NPAD
        for want in (2048, 8192):
            if rem > want * 2:
                sizes.append(want)
                rem -= want
        # near-equal tail segments (512-mult), no runt remainder
        ntail = (rem + 16384 - 1) // 16384
        base = (rem // ntail) // self.SB * self.SB
        extra = (rem - ntail * base) // self.SB
        for i in range(ntail):
            sizes.append(base + self.SB * (1 if i < extra else 0))
        assert sum(sizes) == self.NPAD
        self.SEG_SIZES = sizes
        self.SEG_STARTS = np.concatenate(([0], np.cumsum(sizes))).astype(np.int64)
        self.NSEG = len(sizes)
        self.SBSEG = [sz // self.SB for sz in sizes]  # superblocks per segment
        # filled by prep():
        self.B = None        # [NSEG][T] bucket layout sizes (16-mult, shared)
        self.CHUNKS = None   # total chunks
        self.IDXC = None     # total idx columns (sum B / 16)
        self.IDXC_SEG = None # [NSEG] idx columns per segment
        self.G = None        # NSEG*T gather count


def _prep_edges(cfg: Cfg, row, col):
    """Sort edges per core into (segment, tile) buckets; build tables."""
    M, NC, T, NSEG = cfg.M, cfg.NC, cfg.T, cfg.NSEG
    row = np.asarray(row).astype(np.int64)
    col = np.asarray(col).astype(np.int64)
    seg_starts = cfg.SEG_STARTS

    per_core = []
    counts = np.zeros((M, NSEG, T), dtype=np.int64)
    for m in range(M):
        mask = (row >= m * NC) & (row < (m + 1) * NC)
        r = row[mask] - m * NC
        c = col[mask]
        t = r >> 7
        s = np.searchsorted(seg_starts, c, side="right") - 1
        order = np.lexsort((c, t, s))
        r, c, t, s = r[order], c[order], t[order], s[order]
        np.add.at(counts[m], (s, t), 1)
        per_core.append((r, c, t, s))

    cnt16 = ((counts + 15) // 16) * 16            # per-core 16-mult counts
    B = cnt16.max(axis=0)                          # [NSEG, T] shared layout
    CH = (B + P - 1) // P                          # chunks per bucket
    CHUNKS = int(CH.sum())
    TOT = int(B.sum())
    IDXC = TOT // 16
    IDXC_SEG = (B.sum(axis=1) // 16).astype(np.int64)

    cfg.B = B
    cfg.IDXC = IDXC
    cfg.IDXC_SEG = IDXC_SEG
    # one gather per (segment, tile)
    cfg.G = NSEG * T
    cfg.PAIR_NB = B.copy()
    cfg.PAIR_NCH = (B + P - 1) // P
    cfg.JOBS = {}
    jcol = np.zeros((NSEG, T), dtype=np.int64)  # dest col base per bucket
    dcols = 0
    for s in range(NSEG):
        for t in range(T):
            nch = int(cfg.PAIR_NCH[s, t])
            cfg.JOBS[(s, t)] = [(0, c) for c in range(nch)]
            jcol[s, t] = dcols
            dcols += nch
    cfg.DCOLS = dcols
    cfg.CHUNKS = dcols  # (column count of the dest table)
    cfg.MAXCHP = int(cfg.PAIR_NCH.max())
    cfg.MAXJOBS = max(len(j) for j in cfg.JOBS.values())

    # bucket offsets in slots, (s, t) order (pair-contiguous)
    off = np.zeros((NSEG, T), dtype=np.int64)
    acc = 0
    for s in range(NSEG):
        for t in range(T):
            off[s, t] = acc
            acc += B[s, t]

    idx16 = np.full((M, 16, IDXC), -1, dtype=np.int16)
    dest = np.full((M, P, dcols), -1.0, dtype=np.float32)
    cnts = np.zeros((M, cfg.G), dtype=np.int32)
    for m in range(M):
        r, c, t, s = per_core[m]
        # slot index within each (s,t) bucket, in sorted order
        key = s * T + t
        change = np.flatnonzero(np.diff(key)) + 1
        starts = np.concatenate(([0], change))
        lens = np.diff(np.concatenate((starts, [len(key)])))
        within = np.arange(len(key)) - np.repeat(starts, lens)
        slot = off[s, t] + within
        idx16[m, slot % 16, slot // 16] = (c - seg_starts[s]).astype(np.int16)
        # dest col: bucket-local slot -> (partition, job column)
        jidx = jcol[s, t] + within // P
        dest[m, within % P, jidx] = (r - t * P).astype(np.float32)
        # per-core gather count: own (16-rounded) edge count
        cnts[m] = cnt16[m].reshape(-1)

    idx128 = np.tile(idx16, (1, 8, 1))  # replicate to 128 partitions
    return idx128, dest, cnts


def prep_host(cfg: Cfg, inputs):
    """Build per-core input maps (shared program, per-core data)."""
    x = np.asarray(inputs["x"], dtype=np.float32)
    N = cfg.N
    xT = np.zeros((D, cfg.NPAD), dtype=ml_dtypes.bfloat16)
    xT[:, :N] = x.T.astype(ml_dtypes.bfloat16)

    idx128, dest, cnts = _prep_edges(cfg, inputs["row"], inputs["col"])

    def bcast(v):
        return np.tile(np.asarray(v, np.float32)[None, :], (P, 1))

    att = np.asarray(inputs["att"], np.float32)
    shared = {
        "xT": xT,
        "W0T": np.asarray(inputs["W0"], np.float32).T.astype(ml_dtypes.bfloat16).copy(),
        "W1T": np.asarray(inputs["W1"], np.float32).T.astype(ml_dtypes.bfloat16).copy(),
        "b0c": np.asarray(inputs["b0"], np.float32).reshape(P, 1).copy(),
        "b1c": np.asarray(inputs["b1"], np.float32).reshape(P, 1).copy(),
        "att1b": bcast(att[:D]).copy(),
        "att2c": att[D:].astype(ml_dtypes.bfloat16).reshape(P, 1).copy(),
        "ones_r": np.ones((1, P), dtype=ml_dtypes.bfloat16),
        "ident_bf": np.eye(P, dtype=ml_dtypes.bfloat16),
        "ident_f": np.eye(P, dtype=np.float32),
        "iota_c": np.tile(np.arange(P, dtype=np.float32).astype(ml_dtypes.bfloat16)[None, :], (P, 1)),
        "scale0b": bcast(inputs["scale0"]).copy(),
        "scale1b": bcast(inputs["scale1"]).copy(),
        "off0b": bcast(inputs["offset0"]).copy(),
        "off1b": bcast(inputs["offset1"]).copy(),
    }
    in_maps = []
    for m in range(cfg.M):
        im = dict(shared)
        im["x_ownT"] = np.ascontiguousarray(
            xT[:, m * cfg.NC : m * cfg.NC + cfg.NC_PAD]
        )
        im["idx"] = np.ascontiguousarray(idx128[m])
        im["dest"] = dest[m].astype(ml_dtypes.bfloat16).copy()
        im["cnts"] = cnts[m : m + 1].copy()
        in_maps.append(im)
    return in_maps


def build(nc: bass.Bass, cfg: Cfg, simple_affine: bool):
    """Emit the full program, pipelined per source segment."""
    T, NSEG, SB = cfg.T, cfg.NSEG, cfg.SB
    B = cfg.B
    IDXC_SEG_MAX = int(max(cfg.IDXC_SEG))

    io = {}
    def inp(name, shape, dt):
        io[name] = nc.dram_tensor(name, list(shape), dt, kind="ExternalInput").ap()

    inp("xT", (D, cfg.NPAD), BF16)
    inp("x_ownT", (D, cfg.NC_PAD), BF16)
    inp("W0T", (D, D), BF16)
    inp("W1T", (D, D), BF16)
    inp("b0c", (P, 1), F32)
    inp("b1c", (P, 1), F32)
    inp("att1b", (P, D), F32)
    inp("att2c", (P, 1), BF16)
    inp("ones_r", (1, P), BF16)
    inp("ident_bf", (P, P), BF16)
    inp("ident_f", (P, P), F32)
    inp("iota_c", (P, P), BF16)
    inp("scale0b", (P, D), F32)
    inp("scale1b", (P, D), F32)
    inp("off0b", (P, D), F32)
    inp("off1b", (P, D), F32)
    inp("idx", (P, cfg.IDXC), I16)
    inp("dest", (P, cfg.CHUNKS), BF16)
    inp("cnts", (1, cfg.G), I32)
    out_d = nc.dram_tensor("out", [cfg.NC_PAD, D], F32, kind="ExternalOutput").ap()
    payload = nc.dram_tensor("payload", [cfg.NPAD, 2 * D], BF16, kind="Internal").ap()

    with tile.TileContext(nc) as tc, ExitStack() as ctx:
        singles = ctx.enter_context(tc.tile_pool(name="singles", bufs=1))
        xpool = ctx.enter_context(tc.tile_pool(name="xpool", bufs=3))
        hpool = ctx.enter_context(tc.tile_pool(name="hpool", bufs=3))
        ppool = ctx.enter_context(tc.tile_pool(name="ppool", bufs=3, space="PSUM"))
        pacc = ctx.enter_context(tc.tile_pool(name="pacc", bufs=3, space="PSUM"))
        pepi = ctx.enter_context(tc.tile_pool(name="pepi", bufs=2, space="PSUM"))
        gpool = ctx.enter_context(tc.tile_pool(name="gpool", bufs=10))
        p01pool = ctx.enter_context(tc.tile_pool(name="p01pool", bufs=4))
        ipool = ctx.enter_context(tc.tile_pool(name="ipool", bufs=2))
        epool = ctx.enter_context(tc.tile_pool(name="epool", bufs=4))

        # ---- constants ----
        def load(name, shape, dt):
            t = singles.tile(list(shape), dt, name=f"sb_{name}")
            nc.sync.dma_start(out=t, in_=io[name])
            return t

        W0T_sb = load("W0T", (D, D), BF16)
        W1T_sb = load("W1T", (D, D), BF16)
        b0c_sb = load("b0c", (P, 1), F32)
        b1c_sb = load("b1c", (P, 1), F32)
        att1b_sb = load("att1b", (P, D), F32)
        att2c_sb = load("att2c", (P, 1), BF16)
        ones_sb = load("ones_r", (1, P), BF16)
        dest_sb = load("dest", (P, cfg.CHUNKS), BF16)
        cnts_sb = load("cnts", (1, cfg.G), I32)
        if not simple_affine:
            scale0_sb = load("scale0b", (P, D), F32)
            scale1_sb = load("scale1b", (P, D), F32)
            off0_sb = load("off0b", (P, D), F32)
            off1_sb = load("off1b", (P, D), F32)
            off01_sb = singles.tile([P, D], F32, name="off01")
            nc.vector.tensor_tensor(
                out=off01_sb, in0=off0_sb, in1=off1_sb, op=mybir.AluOpType.add
            )

        ident_bf = load("ident_bf", (P, P), BF16)
        ident_f = load("ident_f", (P, P), F32)
        iota_bf = load("iota_c", (P, P), BF16)
        nc.gpsimd.load_library(library_config.mlp)

        nbreg = nc.alloc_register(mybir.EngineType.Pool, name="nbreg")

        alpha_sb = singles.tile([P, 1], F32, name="alpha_sb")
        nc.vector.memset(alpha_sb, 0.2)
        eps_sb = singles.tile([P, 1], F32, name="eps_sb")
        nc.vector.memset(eps_sb, 1e-9)
        agg_sb = singles.tile([P, T * 2 * D], F32, name="agg_sb")

        # ---- phase 1b superblock: h1 -> payload [U|V] for block i ----
        def emit_sblock(i):
            xb = xpool.tile([P, SB], BF16, name="xb", tag="xb")
            nc.sync.dma_start(out=xb, in_=io["xT"][:, i * SB : (i + 1) * SB])
            ps1 = ppool.tile([P, SB], F32, name="ps1", tag="ps")
            nc.tensor.matmul(out=ps1, lhsT=W1T_sb, rhs=xb, start=True, stop=True)
            h1T = hpool.tile([P, SB], BF16, name="h1T", tag="h1T")
            nc.scalar.activation(
                out=h1T, in_=ps1, func=mybir.ActivationFunctionType.Relu,
                bias=b1c_sb, scale=1.0,
            )
            psw = ppool.tile([1, SB], F32, name="psw", tag="ps")
            nc.tensor.matmul(out=psw, lhsT=att2c_sb, rhs=h1T, start=True, stop=True)
            wrow = hpool.tile([1, SB], BF16, name="wrow", tag="wrow")
            nc.scalar.activation(
                out=wrow, in_=psw, func=mybir.ActivationFunctionType.Prelu,
                scale=1.0, alpha=alpha_sb[0:1, :],
            )
            pswb = ppool.tile([P, SB], F32, name="pswb", tag="ps")
            nc.tensor.matmul(out=pswb, lhsT=ones_sb, rhs=wrow, start=True, stop=True)
            vT = hpool.tile([P, SB], BF16, name="vT", tag="vT")
            nc.vector.tensor_tensor(
                out=vT, in0=h1T, in1=pswb, op=mybir.AluOpType.mult
            )
            psuv = ppool.tile([P, 2 * SB], BF16, name="psuv", tag="ps")
            for j in range(SB // P):
                nc.tensor.transpose(
                    out=psuv[:, j * 256 : j * 256 + 128],
                    in_=h1T[:, j * P : (j + 1) * P], identity=ident_bf,
                )
                nc.tensor.transpose(
                    out=psuv[:, j * 256 + 128 : (j + 1) * 256],
                    in_=vT[:, j * P : (j + 1) * P], identity=ident_bf,
                )
            uv = hpool.tile([P, 2 * SB], BF16, name="uv", tag="uv")
            nc.scalar.copy(out=uv, in_=psuv)
            nc.sync.dma_start(
                out=payload[i * SB : (i + 1) * SB, :].rearrange(
                    "(b p) e -> p b e", p=P
                ),
                in_=uv.rearrange("p (b e) -> p b e", e=2 * D),
            )

        # ---- epilogue for dest tile t: h0 recompute + norms + output ----
        def emit_epilogue(t):
            xo = xpool.tile([P, P], BF16, name="xo", tag="xo")
            nc.sync.dma_start(out=xo, in_=io["x_ownT"][:, t * P : (t + 1) * P])
            ps0 = pepi.tile([P, P], F32, name="ps0", tag="ps0")
            nc.tensor.matmul(out=ps0, lhsT=W0T_sb, rhs=xo, start=True, stop=True)
            h0T = epool.tile([P, P], F32, name="h0T", tag="h0T")
            nc.scalar.activation(
                out=h0T, in_=ps0, func=mybir.ActivationFunctionType.Relu,
                bias=b0c_sb, scale=1.0,
            )
            psT = pepi.tile([P, P], F32, name="psT", tag="ps0")
            nc.tensor.transpose(out=psT, in_=h0T, identity=ident_f)
            h0_t = epool.tile([P, P], F32, name="h0_t", tag="h0_t")
            nc.scalar.copy(out=h0_t, in_=psT)
            tmp = epool.tile([P, P], F32, name="tmp", tag="tmp")
            nc.vector.tensor_tensor(
                out=tmp, in0=h0_t, in1=att1b_sb, op=mybir.AluOpType.mult
            )
            z = epool.tile([P, 1], F32, name="z", tag="z")
            nc.vector.tensor_reduce(
                out=z, in_=tmp, axis=mybir.AxisListType.X, op=mybir.AluOpType.add
            )
            a_col = epool.tile([P, 1], F32, name="a_col", tag="z")
            nc.scalar.activation(
                out=a_col, in_=z,
                func=mybir.ActivationFunctionType.Prelu, scale=1.0, alpha=alpha_sb,
            )
            agg_t = agg_sb[:, t * 2 * D : (t + 1) * 2 * D]
            bagg = epool.tile([P, D], F32, name="bagg", tag="bagg")
            nc.vector.tensor_scalar(
                bagg, agg_t[:, :D], a_col, None, mybir.AluOpType.mult,
            )
            nc.vector.tensor_tensor(
                out=bagg, in0=bagg, in1=agg_t[:, D:], op=mybir.AluOpType.add
            )

            def norm_stats(src, tag):
                st = epool.tile([P, 6], F32, name=f"st{tag}", tag=f"st{tag}")
                nc.vector.bn_stats(out=st, in_=src)
                mv = epool.tile([P, 2], F32, name=f"mv{tag}", tag=f"mv{tag}")
                nc.vector.bn_aggr(out=mv, in_=st)
                rstd = epool.tile([P, 1], F32, name=f"rs{tag}", tag=f"rs{tag}")
                nc.scalar.activation(
                    out=rstd, in_=mv[:, 1:2],
                    func=mybir.ActivationFunctionType.Sqrt, bias=eps_sb,
                )
                nc.vector.reciprocal(out=rstd, in_=rstd)
                return mv[:, 0:1], rstd

            m0, r0 = norm_stats(h0_t, "0")
            m1, r1 = norm_stats(bagg, "1")
            na = epool.tile([P, D], F32, name="na", tag="na")
            nc.vector.tensor_scalar(
                na, h0_t, m0, r0, mybir.AluOpType.subtract, mybir.AluOpType.mult
            )
            nb_ = epool.tile([P, D], F32, name="nb_", tag="nb_")
            nc.vector.tensor_scalar(
                nb_, bagg, m1, r1, mybir.AluOpType.subtract, mybir.AluOpType.mult
            )
            ot = epool.tile([P, D], F32, name="ot", tag="ot")
            if simple_affine:
                nc.vector.tensor_tensor(
                    out=ot, in0=na, in1=nb_, op=mybir.AluOpType.add
                )
            else:
                nc.vector.tensor_tensor(
                    out=na, in0=na, in1=scale0_sb, op=mybir.AluOpType.mult
                )
                nc.vector.tensor_tensor(
                    out=nb_, in0=nb_, in1=scale1_sb, op=mybir.AluOpType.mult
                )
                nc.vector.tensor_tensor(
                    out=na, in0=na, in1=nb_, op=mybir.AluOpType.add
                )
                nc.vector.tensor_tensor(
                    out=ot, in0=na, in1=off01_sb, op=mybir.AluOpType.add
                )
            nc.sync.dma_start(out=out_d[t * P : (t + 1) * P, :], in_=ot)

        # ---- preamble: segment 0 payload + its idx table ----
        idx_tiles = {}
        o16_seg = [0]
        for s in range(NSEG):
            o16_seg.append(o16_seg[-1] + int(cfg.IDXC_SEG[s]))

        def emit_idx_load(s):
            it = ipool.tile([P, IDXC_SEG_MAX], I16, name="idxseg", tag="idxseg")
            w = int(cfg.IDXC_SEG[s])
            nc.sync.dma_start(
                out=it[:, :w], in_=io["idx"][:, o16_seg[s] : o16_seg[s] + w]
            )
            idx_tiles[s] = it

        emit_idx_load(0)
        # pre-zero the rotating gather buffers: slots past a core's own
        # count are never written by the gather, and 0 x NaN garbage
        # would poison the PSUM accumulation
        for _ in range(10):
            gb0 = gpool.tile([P, cfg.MAXCHP * 2 * D], BF16, name="gb", tag="gb")
            nc.vector.memset(gb0, 0.0)
        sb_base = [0]
        for s in range(NSEG):
            sb_base.append(sb_base[-1] + cfg.SBSEG[s])
        for k in range(cfg.SBSEG[0]):
            emit_sblock(k)
        tc.strict_bb_all_engine_barrier()

        # ---- segment loop ----
        gcol = 0  # running chunk column
        for s in range(NSEG):
            sb_next = sb_base[s + 1]  # first superblock of next segment
            sb_quota = cfg.SBSEG[s + 1] if s < NSEG - 1 else 0
            sb_emitted = 0
            seg_base = int(cfg.SEG_STARTS[s])
            seg_len = int(cfg.SEG_SIZES[s])
            o16 = 0  # idx column offset within segment tile
            idx_sb = idx_tiles[s]
            p01_pend = None  # one-hot built for a bucket pair
            for t in range(T):
                nb = int(cfg.PAIR_NB[s][t])
                nch = int(cfg.PAIR_NCH[s][t])
                jobs = cfg.JOBS[(s, t)]
                njobs = len(jobs)
                g = s * T + t
                if nb > 0:
                    nc.gpsimd.reg_load(nbreg, cnts_sb[0:1, g : g + 1])
                    gb = gpool.tile([P, cfg.MAXCHP * 2 * D], BF16, name="gb", tag="gb")
                    nc.gpsimd.dma_gather(
                        out_ap=gb[:, : nch * 256].rearrange(
                            "p (c e) -> p c e", e=256
                        ),
                        in_ap=payload[seg_base : seg_base + seg_len, :],
                        idxs_ap=idx_sb[:, o16 : o16 + nb // 16],
                        num_idxs=nb,
                        num_idxs_reg=nbreg,
                        elem_size=256,
                        single_packet=False,
                        queue_num=g % 4,
                    )
                    o16 += nb // 16
                    # batched one-hot for a pair of buckets in one is_equal:
                    # p01[p, j*128 + f] = (iota[f] == dest[p, base+j])
                    if p01_pend is None:
                        njobs2 = njobs
                        if t + 1 < T:
                            njobs2 += len(cfg.JOBS[(s, t + 1)])
                        p01 = p01pool.tile(
                            [P, 2 * cfg.MAXJOBS * P], BF16, name="p01", tag="p01"
                        )
                        nc.vector.tensor_tensor(
                            out=p01[:, : njobs2 * P].rearrange(
                                "p (c f) -> p c f", f=P
                            ),
                            in0=iota_bf.unsqueeze(1).broadcast_to([P, njobs2, P]),
                            in1=dest_sb[:, gcol : gcol + njobs2]
                            .unsqueeze(2)
                            .broadcast_to([P, njobs2, P]),
                            op=mybir.AluOpType.is_equal,
                        )
                        joff = 0
                        p01_pend = (p01, njobs)  # remaining cols start there
                    else:
                        p01, joff = p01_pend[0], p01_pend[1]
                        p01_pend = None
                    ps_acc = pacc.tile(
                        [P, 2 * D], F32, name="ps_acc", tag="ps_acc"
                    )
                    for k, (_, chunk) in enumerate(jobs):
                        nc.tensor.matmul(
                            out=ps_acc,
                            lhsT=p01[:, (joff + k) * P : (joff + k + 1) * P],
                            rhs=gb[:, chunk * 256 : (chunk + 1) * 256],
                            start=(k == 0),
                            stop=(k == njobs - 1),
                        )
                    agg_t = agg_sb[:, t * 2 * D : (t + 1) * 2 * D]
                    if s == 0:
                        nc.vector.tensor_copy(out=agg_t, in_=ps_acc)
                    else:
                        nc.vector.tensor_tensor(
                            out=agg_t, in0=agg_t, in1=ps_acc,
                            op=mybir.AluOpType.add,
                        )
                    gcol += njobs
                else:
                    p01_pend = None  # keep pairing aligned past empty buckets
                    if s == 0:
                        nc.vector.memset(
                            agg_sb[:, t * 2 * D : (t + 1) * 2 * D], 0.0
                        )

                if s < NSEG - 1:
                    # interleave next segment's payload build + idx load,
                    # paced evenly across this segment's tile loop
                    if t == 1:
                        emit_idx_load(s + 1)
                    while sb_emitted < min(sb_quota, (t + 1) * sb_quota // T):
                        emit_sblock(sb_next + sb_emitted)
                        sb_emitted += 1
                else:
                    emit_epilogue(t)
            if s < NSEG - 1:
                while sb_emitted < sb_quota:
                    emit_sblock(sb_next + sb_emitted)
                    sb_emitted += 1
                tc.strict_bb_all_engine_barrier()
    return io


def make_program(cfg: Cfg, inputs):
    in_maps = prep_host(cfg, inputs)
    simple_affine = (
        np.all(np.asarray(inputs["scale0"]) == 1.0)
        and np.all(np.asarray(inputs["scale1"]) == 1.0)
        and np.all(np.asarray(inputs["offset0"]) == 0.0)
        and np.all(np.asarray(inputs["offset1"]) == 0.0)
    )
    nc = bacc.Bacc(
        "TRN2", target_bir_lowering=False, debug=False, enable_asserts=False,
        num_devices=cfg.M, num_swdge_queues=4,
    )
    build(nc, cfg, bool(simple_affine))
    nc.compile()
    return nc, in_maps


_cache = {}


def kernel(**inputs) -> np.ndarray:
    x = np.asarray(inputs["x"])
    n_nodes = x.shape[0]
    n_cores = 8
    key = hashlib.sha1(
        np.asarray(inputs["row"]).tobytes() + np.asarray(inputs["col"]).tobytes()
    ).hexdigest() + f"_{n_nodes}"
    if key in _cache:
        cfg, nc, _ = _cache[key]
        in_maps = prep_host(cfg, inputs)
    else:
        cfg = Cfg(n_nodes, n_cores)
        nc, in_maps = make_program(cfg, inputs)
        _cache[key] = (cfg, nc, in_maps)

    res = bass_utils.run_bass_kernel_spmd(
        nc, in_maps, core_ids=list(range(n_cores))
    )
    out = np.concatenate(
        [res.results[m]["out"][: cfg.NC] for m in range(n_cores)], axis=0
    )
    return out.astype(np.float32)


# revision 54
# speedup vs baseline: 1.0775x; 1.0016x over previous
"""Trainium2 Bass kernel for nn_AttentionAggregator (GNN message passing).

out = norm(h0)*scale0+offset0 + norm(agg)*scale1+offset1
  h0 = relu(x@W0.T + b0); h1 = relu(x@W1.T + b1)
  a_self = lrelu(h0@att[:d]); a_neigh = lrelu(h1@att[d:])
  agg[i] = sum_{(i,j) in E} (a_self[i]+a_neigh[j]) * h1[j]

Strategy (8 cores, SPMD, no collectives):
  - nodes (rows of output) sharded across cores; edges partitioned by dest row
  - every core recomputes full h1 and writes a node-major payload
    row[j] = [h1[j] | a_neigh[j]*h1[j]] (256 bf16 = 512 B) to its HBM
  - payload build is pipelined per ascending source segment with the
    per-dest-tile dma_gather + one-hot PE segment-sum; gathers alternate
    4 SWDGE queues to overlap descriptor gen with ring drain; per-core
    exact counts via reg_load (trimmed tail idx pads MUST be -1)
  - agg partials accumulate in SBUF f32 across segments; the epilogue
    recomputes h0 per tile and fuses both norms
"""

import hashlib
from contextlib import ExitStack

import numpy as np
import ml_dtypes

import concourse.bass as bass
import concourse.bacc as bacc
import concourse.tile as tile
import concourse.mybir as mybir
from concourse import bass_utils
from concourse import library_config

BF16 = mybir.dt.bfloat16
F32 = mybir.dt.float32
I16 = mybir.dt.int16
I32 = mybir.dt.int32

D = 128  # feature dim (both in and out)
P = 128  # partitions


class Cfg:
    def __init__(self, n_nodes, n_cores):
        assert n_nodes % n_cores == 0
        self.N = n_nodes
        self.M = n_cores
        self.NC = n_nodes // n_cores          # dest rows per core
        self.T = (self.NC + P - 1) // P       # dest tiles per core
        self.NC_PAD = self.T * P
        self.SB = 512                         # phase-1 node superblock
        self.NB = (n_nodes + self.SB - 1) // self.SB
        self.NPAD = self.NB * self.SB
        # ascending source segments (rows); each <= 32768 (int16 idx range),
        # each a multiple of SB. A tiny first segment lets the gathers start
        # almost immediately while later payload builds hide under them.
        sizes = []
        rem = self.NPAD
        for want in (2048, 8192):
            if rem > want * 2:
                sizes.append(want)
                rem -= want
        # near-equal tail segments (512-mult), no runt remainder
        ntail = (rem + 16384 - 1) // 16384
        base = (rem // ntail) // self.SB * self.SB
        extra = (rem - ntail * base) // self.SB
        for i in range(ntail):
            sizes.append(base + self.SB * (1 if i < extra else 0))
        assert sum(sizes) == self.NPAD
        self.SEG_SIZES = sizes
        self.SEG_STARTS = np.concatenate(([0], np.cumsum(sizes))).astype(np.int64)
        self.NSEG = len(sizes)
        self.SBSEG = [sz // self.SB for sz in sizes]  # superblocks per segment
        # filled by prep():
        self.B = None        # [NSEG][T] bucket layout sizes (16-mult, shared)
        self.CHUNKS = None   # total chunks
        self.IDXC = None     # total idx columns (sum B / 16)
        self.IDXC_SEG = None # [NSEG] idx columns per segment
        self.G = None        # NSEG*T gather count


def _prep_edges(cfg: Cfg, row, col):
    """Sort edges per core into (segment, tile) buckets; build tables."""
    M, NC, T, NSEG = cfg.M, cfg.NC, cfg.T, cfg.NSEG
    row = np.asarray(row).astype(np.int64)
    col = np.asarray(col).astype(np.int64)
    seg_starts = cfg.SEG_STARTS

    per_core = []
    counts = np.zeros((M, NSEG, T), dtype=np.int64)
    for m in range(M):
        mask = (row >= m * NC) & (row < (m + 1) * NC)
        r = row[mask] - m * NC
        c = col[mask]
        t = r >> 7
        s = np.searchsorted(seg_starts, c, side="right") - 1
        order = np.lexsort((c, t, s))
        r, c, t, s = r[order], c[order], t[order], s[order]
        np.add.at(counts[m], (s, t), 1)
        per_core.append((r, c, t, s))

    cnt16 = ((counts + 15) // 16) * 16            # per-core 16-mult counts
    B = cnt16.max(axis=0)                          # [NSEG, T] shared layout
    CH = (B + P - 1) // P                          # chunks per bucket
    CHUNKS = int(CH.sum())
    TOT = int(B.sum())
    IDXC = TOT // 16
    IDXC_SEG = (B.sum(axis=1) // 16).astype(np.int64)

    cfg.B = B
    cfg.IDXC = IDXC
    cfg.IDXC_SEG = IDXC_SEG
    # one gather per (segment, tile)
    cfg.G = NSEG * T
    cfg.PAIR_NB = B.copy()
    cfg.PAIR_NCH = (B + P - 1) // P
    cfg.JOBS = {}
    jcol = np.zeros((NSEG, T), dtype=np.int64)  # dest col base per bucket
    dcols = 0
    for s in range(NSEG):
        for t in range(T):
            nch = int(cfg.PAIR_NCH[s, t])
            cfg.JOBS[(s, t)] = [(0, c) for c in range(nch)]
            jcol[s, t] = dcols
            dcols += nch
    cfg.DCOLS = dcols
    cfg.CHUNKS = dcols  # (column count of the dest table)
    cfg.MAXCHP = int(cfg.PAIR_NCH.max())
    cfg.MAXJOBS = max(len(j) for j in cfg.JOBS.values())

    # bucket offsets in slots, (s, t) order (pair-contiguous)
    off = np.zeros((NSEG, T), dtype=np.int64)
    acc = 0
    for s in range(NSEG):
        for t in range(T):
            off[s, t] = acc
            acc += B[s, t]

    idx16 = np.full((M, 16, IDXC), -1, dtype=np.int16)
    dest = np.full((M, P, dcols), -1.0, dtype=np.float32)
    cnts = np.zeros((M, cfg.G), dtype=np.int32)
    for m in range(M):
        r, c, t, s = per_core[m]
        # slot index within each (s,t) bucket, in sorted order
        key = s * T + t
        change = np.flatnonzero(np.diff(key)) + 1
        starts = np.concatenate(([0], change))
        lens = np.diff(np.concatenate((starts, [len(key)])))
        within = np.arange(len(key)) - np.repeat(starts, lens)
        slot = off[s, t] + within
        idx16[m, slot % 16, slot // 16] = (c - seg_starts[s]).astype(np.int16)
        # dest col: bucket-local slot -> (partition, job column)
        jidx = jcol[s, t] + within // P
        dest[m, within % P, jidx] = (r - t * P).astype(np.float32)
        # per-core gather count: own (16-rounded) edge count
        cnts[m] = cnt16[m].reshape(-1)

    idx128 = np.tile(idx16, (1, 8, 1))  # replicate to 128 partitions
    return idx128, dest, cnts


def prep_host(cfg: Cfg, inputs):
    """Build per-core input maps (shared program, per-core data)."""
    x = np.asarray(inputs["x"], dtype=np.float32)
    N = cfg.N
    xT = np.zeros((D, cfg.NPAD), dtype=ml_dtypes.bfloat16)
    xT[:, :N] = x.T.astype(ml_dtypes.bfloat16)

    idx128, dest, cnts = _prep_edges(cfg, inputs["row"], inputs["col"])

    def bcast(v):
        return np.tile(np.asarray(v, np.float32)[None, :], (P, 1))

    att = np.asarray(inputs["att"], np.float32)
    shared = {
        "xT": xT,
        "W0T": np.asarray(inputs["W0"], np.float32).T.astype(ml_dtypes.bfloat16).copy(),
        "W1T": np.asarray(inputs["W1"], np.float32).T.astype(ml_dtypes.bfloat16).copy(),
        "b0c": np.asarray(inputs["b0"], np.float32).reshape(P, 1).copy(),
        "b1c": np.asarray(inputs["b1"], np.float32).reshape(P, 1).copy(),
        "att1b": bcast(att[:D]).copy(),
        "att2c": att[D:].astype(ml_dtypes.bfloat16).reshape(P, 1).copy(),
        "ones_r": np.ones((1, P), dtype=ml_dtypes.bfloat16),
        "ident_bf": np.eye(P, dtype=ml_dtypes.bfloat16),
        "ident_f": np.eye(P, dtype=np.float32),
        "iota_c": np.tile(np.arange(P, dtype=np.float32).astype(ml_dtypes.bfloat16)[None, :], (P, 1)),
        "iota_w": np.tile(np.arange(P, dtype=np.float32).astype(ml_dtypes.bfloat16)[None, :], (P, 2 * cfg.MAXJOBS)),
        "scale0b": bcast(inputs["scale0"]).copy(),
        "scale1b": bcast(inputs["scale1"]).copy(),
        "off0b": bcast(inputs["offset0"]).copy(),
        "off1b": bcast(inputs["offset1"]).copy(),
    }
    in_maps = []
    for m in range(cfg.M):
        im = dict(shared)
        im["x_ownT"] = np.ascontiguousarray(
            xT[:, m * cfg.NC : m * cfg.NC + cfg.NC_PAD]
        )
        im["idx"] = np.ascontiguousarray(idx128[m])
        im["dest"] = dest[m].astype(ml_dtypes.bfloat16).copy()
        im["cnts"] = cnts[m : m + 1].copy()
        in_maps.append(im)
    return in_maps


def build(nc: bass.Bass, cfg: Cfg, simple_affine: bool):
    """Emit the full program, pipelined per source segment."""
    T, NSEG, SB = cfg.T, cfg.NSEG, cfg.SB
    B = cfg.B
    IDXC_SEG_MAX = int(max(cfg.IDXC_SEG))

    io = {}
    def inp(name, shape, dt):
        io[name] = nc.dram_tensor(name, list(shape), dt, kind="ExternalInput").ap()

    inp("xT", (D, cfg.NPAD), BF16)
    inp("x_ownT", (D, cfg.NC_PAD), BF16)
    inp("W0T", (D, D), BF16)
    inp("W1T", (D, D), BF16)
    inp("b0c", (P, 1), F32)
    inp("b1c", (P, 1), F32)
    inp("att1b", (P, D), F32)
    inp("att2c", (P, 1), BF16)
    inp("ones_r", (1, P), BF16)
    inp("ident_bf", (P, P), BF16)
    inp("ident_f", (P, P), F32)
    inp("iota_c", (P, P), BF16)
    inp("iota_w", (P, 2 * cfg.MAXJOBS * P), BF16)
    inp("scale0b", (P, D), F32)
    inp("scale1b", (P, D), F32)
    inp("off0b", (P, D), F32)
    inp("off1b", (P, D), F32)
    inp("idx", (P, cfg.IDXC), I16)
    inp("dest", (P, cfg.CHUNKS), BF16)
    inp("cnts", (1, cfg.G), I32)
    out_d = nc.dram_tensor("out", [cfg.NC_PAD, D], F32, kind="ExternalOutput").ap()
    payload = nc.dram_tensor("payload", [cfg.NPAD, 2 * D], BF16, kind="Internal").ap()

    with tile.TileContext(nc) as tc, ExitStack() as ctx:
        singles = ctx.enter_context(tc.tile_pool(name="singles", bufs=1))
        xpool = ctx.enter_context(tc.tile_pool(name="xpool", bufs=3))
        hpool = ctx.enter_context(tc.tile_pool(name="hpool", bufs=3))
        ppool = ctx.enter_context(tc.tile_pool(name="ppool", bufs=3, space="PSUM"))
        pacc = ctx.enter_context(tc.tile_pool(name="pacc", bufs=3, space="PSUM"))
        pepi = ctx.enter_context(tc.tile_pool(name="pepi", bufs=2, space="PSUM"))
        gpool = ctx.enter_context(tc.tile_pool(name="gpool", bufs=10))
        p01pool = ctx.enter_context(tc.tile_pool(name="p01pool", bufs=4))
        ipool = ctx.enter_context(tc.tile_pool(name="ipool", bufs=2))
        epool = ctx.enter_context(tc.tile_pool(name="epool", bufs=4))

        # ---- constants ----
        def load(name, shape, dt):
            t = singles.tile(list(shape), dt, name=f"sb_{name}")
            nc.sync.dma_start(out=t, in_=io[name])
            return t

        W0T_sb = load("W0T", (D, D), BF16)
        W1T_sb = load("W1T", (D, D), BF16)
        b0c_sb = load("b0c", (P, 1), F32)
        b1c_sb = load("b1c", (P, 1), F32)
        att1b_sb = load("att1b", (P, D), F32)
        att2c_sb = load("att2c", (P, 1), BF16)
        ones_sb = load("ones_r", (1, P), BF16)
        dest_sb = load("dest", (P, cfg.CHUNKS), BF16)
        cnts_sb = load("cnts", (1, cfg.G), I32)
        if not simple_affine:
            scale0_sb = load("scale0b", (P, D), F32)
            scale1_sb = load("scale1b", (P, D), F32)
            off0_sb = load("off0b", (P, D), F32)
            off1_sb = load("off1b", (P, D), F32)
            off01_sb = singles.tile([P, D], F32, name="off01")
            nc.vector.tensor_tensor(
                out=off01_sb, in0=off0_sb, in1=off1_sb, op=mybir.AluOpType.add
            )

        ident_bf = load("ident_bf", (P, P), BF16)
        ident_f = load("ident_f", (P, P), F32)
        iota_bf = load("iota_c", (P, P), BF16)
        iota_w = load("iota_w", (P, 2 * cfg.MAXJOBS * P), BF16)
        nc.gpsimd.load_library(library_config.mlp)

        nbreg = nc.alloc_register(mybir.EngineType.Pool, name="nbreg")

        alpha_sb = singles.tile([P, 1], F32, name="alpha_sb")
        nc.vector.memset(alpha_sb, 0.2)
        eps_sb = singles.tile([P, 1], F32, name="eps_sb")
        nc.vector.memset(eps_sb, 1e-9)
        agg_sb = singles.tile([P, T * 2 * D], F32, name="agg_sb")

        # ---- phase 1b superblock: h1 -> payload [U|V] for block i ----
        def emit_sblock(i):
            xb = xpool.tile([P, SB], BF16, name="xb", tag="xb")
            nc.sync.dma_start(out=xb, in_=io["xT"][:, i * SB : (i + 1) * SB])
            ps1 = ppool.tile([P, SB], F32, name="ps1", tag="ps")
            nc.tensor.matmul(out=ps1, lhsT=W1T_sb, rhs=xb, start=True, stop=True)
            h1T = hpool.tile([P, SB], BF16, name="h1T", tag="h1T")
            nc.scalar.activation(
                out=h1T, in_=ps1, func=mybir.ActivationFunctionType.Relu,
                bias=b1c_sb, scale=1.0,
            )
            psw = ppool.tile([1, SB], F32, name="psw", tag="ps")
            nc.tensor.matmul(out=psw, lhsT=att2c_sb, rhs=h1T, start=True, stop=True)
            wrow = hpool.tile([1, SB], BF16, name="wrow", tag="wrow")
            nc.scalar.activation(
                out=wrow, in_=psw, func=mybir.ActivationFunctionType.Prelu,
                scale=1.0, alpha=alpha_sb[0:1, :],
            )
            pswb = ppool.tile([P, SB], F32, name="pswb", tag="ps")
            nc.tensor.matmul(out=pswb, lhsT=ones_sb, rhs=wrow, start=True, stop=True)
            vT = hpool.tile([P, SB], BF16, name="vT", tag="vT")
            nc.vector.tensor_tensor(
                out=vT, in0=h1T, in1=pswb, op=mybir.AluOpType.mult
            )
            psuv = ppool.tile([P, 2 * SB], BF16, name="psuv", tag="ps")
            for j in range(SB // P):
                nc.tensor.transpose(
                    out=psuv[:, j * 256 : j * 256 + 128],
                    in_=h1T[:, j * P : (j + 1) * P], identity=ident_bf,
                )
                nc.tensor.transpose(
                    out=psuv[:, j * 256 + 128 : (j + 1) * 256],
                    in_=vT[:, j * P : (j + 1) * P], identity=ident_bf,
                )
            uv = hpool.tile([P, 2 * SB], BF16, name="uv", tag="uv")
            nc.scalar.copy(out=uv, in_=psuv)
            nc.sync.dma_start(
                out=payload[i * SB : (i + 1) * SB, :].rearrange(
                    "(b p) e -> p b e", p=P
                ),
                in_=uv.rearrange("p (b e) -> p b e", e=2 * D),
            )

        # ---- epilogue for dest tile t: h0 recompute + norms + output ----
        def emit_epilogue(t):
            xo = xpool.tile([P, P], BF16, name="xo", tag="xo")
            nc.sync.dma_start(out=xo, in_=io["x_ownT"][:, t * P : (t + 1) * P])
            ps0 = pepi.tile([P, P], F32, name="ps0", tag="ps0")
            nc.tensor.matmul(out=ps0, lhsT=W0T_sb, rhs=xo, start=True, stop=True)
            h0T = epool.tile([P, P], F32, name="h0T", tag="h0T")
            nc.scalar.activation(
                out=h0T, in_=ps0, func=mybir.ActivationFunctionType.Relu,
                bias=b0c_sb, scale=1.0,
            )
            psT = pepi.tile([P, P], F32, name="psT", tag="ps0")
            nc.tensor.transpose(out=psT, in_=h0T, identity=ident_f)
            h0_t = epool.tile([P, P], F32, name="h0_t", tag="h0_t")
            nc.scalar.copy(out=h0_t, in_=psT)
            tmp = epool.tile([P, P], F32, name="tmp", tag="tmp")
            nc.vector.tensor_tensor(
                out=tmp, in0=h0_t, in1=att1b_sb, op=mybir.AluOpType.mult
            )
            z = epool.tile([P, 1], F32, name="z", tag="z")
            nc.vector.tensor_reduce(
                out=z, in_=tmp, axis=mybir.AxisListType.X, op=mybir.AluOpType.add
            )
            a_col = epool.tile([P, 1], F32, name="a_col", tag="z")
            nc.scalar.activation(
                out=a_col, in_=z,
                func=mybir.ActivationFunctionType.Prelu, scale=1.0, alpha=alpha_sb,
            )
            agg_t = agg_sb[:, t * 2 * D : (t + 1) * 2 * D]
            bagg = epool.tile([P, D], F32, name="bagg", tag="bagg")
            nc.vector.tensor_scalar(
                bagg, agg_t[:, :D], a_col, None, mybir.AluOpType.mult,
            )
            nc.vector.tensor_tensor(
                out=bagg, in0=bagg, in1=agg_t[:, D:], op=mybir.AluOpType.add
            )

            def norm_stats(src, tag):
                st = epool.tile([P, 6], F32, name=f"st{tag}", tag=f"st{tag}")
                nc.vector.bn_stats(out=st, in_=src)
                mv = epool.tile([P, 2], F32, name=f"mv{tag}", tag=f"mv{tag}")
                nc.vector.bn_aggr(out=mv, in_=st)
                rstd = epool.tile([P, 1], F32, name=f"rs{tag}", tag=f"rs{tag}")
                nc.scalar.activation(
                    out=rstd, in_=mv[:, 1:2],
                    func=mybir.ActivationFunctionType.Sqrt, bias=eps_sb,
                )
                nc.vector.reciprocal(out=rstd, in_=rstd)
                return mv[:, 0:1], rstd

            m0, r0 = norm_stats(h0_t, "0")
            m1, r1 = norm_stats(bagg, "1")
            na = epool.tile([P, D], F32, name="na", tag="na")
            nc.vector.tensor_scalar(
                na, h0_t, m0, r0, mybir.AluOpType.subtract, mybir.AluOpType.mult
            )
            nb_ = epool.tile([P, D], F32, name="nb_", tag="nb_")
            nc.vector.tensor_scalar(
                nb_, bagg, m1, r1, mybir.AluOpType.subtract, mybir.AluOpType.mult
            )
            ot = epool.tile([P, D], F32, name="ot", tag="ot")
            if simple_affine:
                nc.vector.tensor_tensor(
                    out=ot, in0=na, in1=nb_, op=mybir.AluOpType.add
                )
            else:
                nc.vector.tensor_tensor(
                    out=na, in0=na, in1=scale0_sb, op=mybir.AluOpType.mult
                )
                nc.vector.tensor_tensor(
                    out=nb_, in0=nb_, in1=scale1_sb, op=mybir.AluOpType.mult
                )
                nc.vector.tensor_tensor(
                    out=na, in0=na, in1=nb_, op=mybir.AluOpType.add
                )
                nc.vector.tensor_tensor(
                    out=ot, in0=na, in1=off01_sb, op=mybir.AluOpType.add
                )
            nc.sync.dma_start(out=out_d[t * P : (t + 1) * P, :], in_=ot)

        # ---- preamble: segment 0 payload + its idx table ----
        idx_tiles = {}
        o16_seg = [0]
        for s in range(NSEG):
            o16_seg.append(o16_seg[-1] + int(cfg.IDXC_SEG[s]))

        def emit_idx_load(s):
            it = ipool.tile([P, IDXC_SEG_MAX], I16, name="idxseg", tag="idxseg")
            w = int(cfg.IDXC_SEG[s])
            nc.sync.dma_start(
                out=it[:, :w], in_=io["idx"][:, o16_seg[s] : o16_seg[s] + w]
            )
            idx_tiles[s] = it

        emit_idx_load(0)
        # pre-zero the rotating gather buffers: slots past a core's own
        # count are never written by the gather, and 0 x NaN garbage
        # would poison the PSUM accumulation
        for _ in range(10):
            gb0 = gpool.tile([P, cfg.MAXCHP * 2 * D], BF16, name="gb", tag="gb")
            nc.vector.memset(gb0, 0.0)
        sb_base = [0]
        for s in range(NSEG):
            sb_base.append(sb_base[-1] + cfg.SBSEG[s])
        for k in range(cfg.SBSEG[0]):
            emit_sblock(k)
        tc.strict_bb_all_engine_barrier()

        # ---- segment loop ----
        gcol = 0  # running chunk column
        for s in range(NSEG):
            sb_next = sb_base[s + 1]  # first superblock of next segment
            sb_quota = cfg.SBSEG[s + 1] if s < NSEG - 1 else 0
            sb_emitted = 0
            seg_base = int(cfg.SEG_STARTS[s])
            seg_len = int(cfg.SEG_SIZES[s])
            o16 = 0  # idx column offset within segment tile
            idx_sb = idx_tiles[s]
            p01_pend = None  # one-hot built for a bucket pair
            for t in range(T):
                nb = int(cfg.PAIR_NB[s][t])
                nch = int(cfg.PAIR_NCH[s][t])
                jobs = cfg.JOBS[(s, t)]
                njobs = len(jobs)
                g = s * T + t
                if nb > 0:
                    nc.gpsimd.reg_load(nbreg, cnts_sb[0:1, g : g + 1])
                    gb = gpool.tile([P, cfg.MAXCHP * 2 * D], BF16, name="gb", tag="gb")
                    nc.gpsimd.dma_gather(
                        out_ap=gb[:, : nch * 256].rearrange(
                            "p (c e) -> p c e", e=256
                        ),
                        in_ap=payload[seg_base : seg_base + seg_len, :],
                        idxs_ap=idx_sb[:, o16 : o16 + nb // 16],
                        num_idxs=nb,
                        num_idxs_reg=nbreg,
                        elem_size=256,
                        single_packet=False,
                        queue_num=g % 4,
                    )
                    o16 += nb // 16
                    # batched one-hot for a pair of buckets in one is_equal:
                    # p01[p, j*128 + f] = (iota[f] == dest[p, base+j])
                    if p01_pend is None:
                        njobs2 = njobs
                        if t + 1 < T:
                            njobs2 += len(cfg.JOBS[(s, t + 1)])
                        p01 = p01pool.tile(
                            [P, 2 * cfg.MAXJOBS * P], BF16, name="p01", tag="p01"
                        )
                        nc.vector.tensor_tensor(
                            out=p01[:, : njobs2 * P].rearrange(
                                "p (c f) -> p c f", f=P
                            ),
                            in0=iota_w[:, : njobs2 * P].rearrange(
                                "p (c f) -> p c f", f=P
                            ),
                            in1=dest_sb[:, gcol : gcol + njobs2]
                            .unsqueeze(2)
                            .broadcast_to([P, njobs2, P]),
                            op=mybir.AluOpType.is_equal,
                        )
                        joff = 0
                        p01_pend = (p01, njobs)  # remaining cols start there
                    else:
                        p01, joff = p01_pend[0], p01_pend[1]
                        p01_pend = None
                    ps_acc = pacc.tile(
                        [P, 2 * D], F32, name="ps_acc", tag="ps_acc"
                    )
                    for k, (_, chunk) in enumerate(jobs):
                        nc.tensor.matmul(
                            out=ps_acc,
                            lhsT=p01[:, (joff + k) * P : (joff + k + 1) * P],
                            rhs=gb[:, chunk * 256 : (chunk + 1) * 256],
                            start=(k == 0),
                            stop=(k == njobs - 1),
                        )
                    agg_t = agg_sb[:, t * 2 * D : (t + 1) * 2 * D]
                    if s == 0:
                        nc.vector.tensor_copy(out=agg_t, in_=ps_acc)
                    else:
                        nc.vector.tensor_tensor(
                            out=agg_t, in0=agg_t, in1=ps_acc,
                            op=mybir.AluOpType.add,
                        )
                    gcol += njobs
                else:
                    p01_pend = None  # keep pairing aligned past empty buckets
                    if s == 0:
                        nc.vector.memset(
                            agg_sb[:, t * 2 * D : (t + 1) * 2 * D], 0.0
                        )

                if s < NSEG - 1:
                    # interleave next segment's payload build + idx load,
                    # paced evenly across this segment's tile loop
                    if t == 1:
                        emit_idx_load(s + 1)
                    while sb_emitted < min(sb_quota, (t + 1) * sb_quota // T):
                        emit_sblock(sb_next + sb_emitted)
                        sb_emitted += 1
                else:
                    emit_epilogue(t)
            if s < NSEG - 1:
                while sb_emitted < sb_quota:
                    emit_sblock(sb_next + sb_emitted)
                    sb_emitted += 1
                tc.strict_bb_all_engine_barrier()
    return io


def make_program(cfg: Cfg, inputs):
    in_maps = prep_host(cfg, inputs)
    simple_affine = (
        np.all(np.asarray(inputs["scale0"]) == 1.0)
        and np.all(np.asarray(inputs["scale1"]) == 1.0)
        and np.all(np.asarray(inputs["offset0"]) == 0.0)
        and np.all(np.asarray(inputs["offset1"]) == 0.0)
    )
    nc = bacc.Bacc(
        "TRN2", target_bir_lowering=False, debug=False, enable_asserts=False,
        num_devices=cfg.M, num_swdge_queues=4,
    )
    build(nc, cfg, bool(simple_affine))
    nc.compile()
    return nc, in_maps


_cache = {}


def kernel(**inputs) -> np.ndarray:
    x = np.asarray(inputs["x"])
    n_nodes = x.shape[0]
    n_cores = 8
    key = hashlib.sha1(
        np.asarray(inputs["row"]).tobytes() + np.asarray(inputs["col"]).tobytes()
    ).hexdigest() + f"_{n_nodes}"
    if key in _cache:
        cfg, nc, _ = _cache[key]
        in_maps = prep_host(cfg, inputs)
    else:
        cfg = Cfg(n_nodes, n_cores)
        nc, in_maps = make_program(cfg, inputs)
        _cache[key] = (cfg, nc, in_maps)

    res = bass_utils.run_bass_kernel_spmd(
        nc, in_maps, core_ids=list(range(n_cores))
    )
    out = np.concatenate(
        [res.results[m]["out"][: cfg.NC] for m in range(n_cores)], axis=0
    )
    return out.astype(np.float32)
